# revision 6
# baseline (speedup 1.0000x reference)
"""Trainium2 Bass kernel for nn_DeepCAD (3x GNN-attention layers -> 2x LSTM -> 4 MLP heads).

Sharding: data-parallel over batch B=8 across the 8 NeuronCores (one batch
element per core); small weights replicated; host does the scatter/gather.

Per-core layout strategy:
  - GNN runs in "transposed" space: hT [H=128 partitions, N=1024 free].
  - attention: attT[j, i] = sigmoid(sj[j] + si[i] + ab) * adjT[j, i], built
    per 128-row chunk of j with sigmoid-bias = per-partition sj column and a
    PE-broadcast si row; att @ h via PE with lhsT = h (normal orientation,
    obtained by PE transposes of hT).
  - LSTM: x-parts precomputed as one big matmul into an interleaved
    [128, 4t+g] SBUF buffer; the 1024-step scan runs fully unrolled with
    gates-on-partitions [128, 4] tiles; gate col order (i, f, o, g) so one
    Sigmoid covers i,f,o and one Tanh covers g. Layer 1 is chunk-pipelined
    behind layer 0 (wavefront).
  - heads: chunked matmuls + PE transposes back to [t, dout] rows.
"""

import os
import numpy as np

import concourse.bass as bass
import concourse.tile as tile
from concourse import mybir
from concourse import bass_utils
from concourse.vector_clock import ScopedClock, VectorClock

F32 = mybir.dt.float32
AF = mybir.ActivationFunctionType
OP = mybir.AluOpType

B, N, ND, H = 8, 1024, 64, 128
NUM_OPS, NUM_PARAMS, SKETCH = 64, 256, 128
NCORES = 8

# gate permutation: torch order (i, f, g, o) -> our col order (i, f, o, g)
_GATE_PERM = np.concatenate([
    np.arange(0, 128),       # i
    np.arange(128, 256),     # f
    np.arange(384, 512),     # o
    np.arange(256, 384),     # g
])


# ---------------------------------------------------------------------------
# Workaround: this walrus encodes at most ONE sem-wait per Drain instruction.
# Tile's exit drain waits on every allocated sem at once -> split it.
def _drain_and_barrier_split(self, tick_clock, wait_clock):
    nc = self.nc
    vclock = tick_clock.global_clock
    emitted = False
    n = len(vclock)
    for p in range(n):
        t = vclock[p]
        if t <= 0:
            continue
        vc = VectorClock([0] * n)
        vc.require_at_least(p, t)
        d = nc.sync.drain()
        wait_clock.add_sem_waits(d.ins, ScopedClock({None: vc}))
        emitted = True
    if not emitted:
        nc.sync.drain()

    nc.all_engine_barrier()
    assert self.sems is not None
    popped = nc._tile_sem_poison_stack.pop()
    assert popped is self._sem_poison
    nc.clear_and_free_semaphores(list(self.sems.allocated().values()))
    nc.all_engine_barrier()


def _apply_tile_patch():
    tile.TileContext._drain_and_barrier = _drain_and_barrier_split


def _split_multi_waits(nc, limit=1):
    """This walrus encodes at most `limit` sem-waits per instruction; hoist
    extra waits onto same-engine NoOps inserted just before the instruction."""
    total = 0
    for f in nc.m.functions:
        for bb in f.blocks:
            insts = bb.instructions
            new_list = []
            for inst in insts:
                si = inst.sync_info
                if si is not None:
                    waits = list(si.on_wait)
                    if len(waits) > limit:
                        for w in waits[:-limit]:
                            nop = mybir.InstNoOp(name=f"waitsplit-{total}")
                            total += 1
                            nop.engine = inst.engine
                            nop.sync_info = mybir.SyncInfo(
                                on_wait=[w], on_update=[])
                            new_list.append(nop)
                        inst.sync_info = mybir.SyncInfo(
                            on_wait=waits[-limit:],
                            on_update=list(si.on_update))
                new_list.append(inst)
            if len(new_list) != len(insts):
                insts[:] = new_list
    return total


# ---------------------------------------------------------------------------
def build_program(n=N):
    """Build the per-core Bass program (SPMD: same program, per-core data)."""
    _apply_tile_patch()
    nc = bass.Bass("TRN2", target_bir_lowering=False, debug=False)

    nch = n // 128          # GNN j-chunks
    MM = min(512, n)        # moving-operand matmul width
    CH = min(128, n)        # LSTM emission chunk
    nlch = n // CH          # LSTM chunks
    HC = min(512, n)        # head chunk
    nhc = n // HC

    din0 = ND

    # ---- DRAM tensors -----------------------------------------------------
    def din_(name, shape):
        return nc.dram_tensor(name, list(shape), F32, kind="ExternalInput")

    xT_d = din_("xT", [din0, n])
    adjT_d = din_("adjT", [n, n])
    ident_d = din_("ident", [128, 128])

    gw_d, gb_d, gaw_d, gab_d = [], [], [], []
    for l in range(3):
        din = din0 if l == 0 else H
        gw_d.append(din_(f"g{l}_w", [din, H]))
        gb_d.append(din_(f"g{l}_b", [H, 1]))
        gaw_d.append(din_(f"g{l}_aw", [H, 2]))
        gab_d.append(din_(f"g{l}_ab", [H, 1]))

    wihT_d, whhT_d, bsum_d = [], [], []
    for l in range(2):
        wihT_d.append(din_(f"l{l}_wihT", [H, 4 * H]))
        whhT_d.append(din_(f"l{l}_whhT", [H, 4 * H]))
        bsum_d.append(din_(f"l{l}_bsum", [H, 4]))

    HEADS = [("op", NUM_OPS), ("pa", NUM_PARAMS), ("sk", SKETCH), ("no", ND)]
    hw1_d, hb1_d, hw2_d, hb2_d, hout_d = {}, {}, {}, {}, {}
    for hn, dout in HEADS:
        hw1_d[hn] = din_(f"{hn}_w1", [H, H])
        hb1_d[hn] = din_(f"{hn}_b1", [H, 1])
        hw2_d[hn] = din_(f"{hn}_w2", [H, dout])
        nblk = (dout + 127) // 128
        hb2_d[hn] = din_(f"{hn}_b2", [min(dout, 128), nblk])
        hout_d[hn] = nc.dram_tensor(f"{hn}_out", [n, dout], F32,
                                    kind="ExternalOutput")

    with tile.TileContext(nc) as tc:
        with (
            tc.tile_pool(name="const", bufs=1) as const,
            tc.tile_pool(name="work", bufs=2) as work,
            tc.tile_pool(name="att_p", bufs=3) as att_p,
            tc.tile_pool(name="small", bufs=4) as small,
        ):
            # ---- load constants/weights ----------------------------------
            ident = const.tile([128, 128], F32)
            nc.sync.dma_start(out=ident, in_=ident_d[:, :])
            ones_row = const.tile([1, 128], F32)
            nc.vector.memset(ones_row, 1.0)

            adj_sb = const.tile([128, nch, n], F32)
            for jc in range(nch):
                nc.sync.dma_start(out=adj_sb[:, jc, :],
                                  in_=adjT_d[jc * 128:(jc + 1) * 128, :])

            xT_sb = const.tile([din0, n], F32)
            nc.sync.dma_start(out=xT_sb, in_=xT_d[:, :])

            gw, gb, gaw, gab = [], [], [], []
            for l in range(3):
                din = din0 if l == 0 else H
                w = const.tile([din, H], F32, name=f"gw{l}")
                nc.sync.dma_start(out=w, in_=gw_d[l][:, :])
                b = const.tile([H, 1], F32, name=f"gb{l}")
                nc.sync.dma_start(out=b, in_=gb_d[l][:, :])
                aw = const.tile([H, 2], F32, name=f"gaw{l}")
                nc.sync.dma_start(out=aw, in_=gaw_d[l][:, :])
                ab = const.tile([H, 1], F32, name=f"gab{l}")
                nc.sync.dma_start(out=ab, in_=gab_d[l][:, :])
                gw.append(w); gb.append(b); gaw.append(aw); gab.append(ab)

            wihT, whhT, bsum = [], [], []
            for l in range(2):
                a = const.tile([H, 4 * H], F32, name=f"wihT{l}")
                nc.sync.dma_start(out=a, in_=wihT_d[l][:, :])
                b_ = const.tile([H, 4 * H], F32, name=f"whhT{l}")
                nc.sync.dma_start(out=b_, in_=whhT_d[l][:, :])
                c_ = const.tile([H, 4], F32, name=f"bsum{l}")
                nc.sync.dma_start(out=c_, in_=bsum_d[l][:, :])
                wihT.append(a); whhT.append(b_); bsum.append(c_)

            hw1, hb1, hw2, hb2 = {}, {}, {}, {}
            for hn, dout in HEADS:
                nblk = (dout + 127) // 128
                hw1[hn] = const.tile([H, H], F32, name=f"hw1{hn}")
                nc.sync.dma_start(out=hw1[hn], in_=hw1_d[hn][:, :])
                hb1[hn] = const.tile([H, 1], F32, name=f"hb1{hn}")
                nc.sync.dma_start(out=hb1[hn], in_=hb1_d[hn][:, :])
                hw2[hn] = const.tile([H, dout], F32, name=f"hw2{hn}")
                nc.sync.dma_start(out=hw2[hn], in_=hw2_d[hn][:, :])
                hb2[hn] = const.tile([min(dout, 128), nblk], F32,
                                     name=f"hb2{hn}")
                nc.sync.dma_start(out=hb2[hn], in_=hb2_d[hn][:, :])

            # ---- GNN phase (own PSUM scope) ------------------------------
            cur = xT_sb
            din = din0
            hT3 = None
            with (
                tc.tile_pool(name="gps_big", bufs=2, space="PSUM") as gps_big,
                tc.tile_pool(name="gps_misc", bufs=2, space="PSUM") as gps_misc,
            ):
                for l in range(3):
                    # hT = W.T @ cur + b
                    hps = gps_big.tile([128, n], F32, tag="big")
                    for h0 in range(0, n, MM):
                        nc.tensor.matmul(hps[:, h0:h0 + MM], gw[l][:din, :],
                                         cur[:din, h0:h0 + MM],
                                         start=True, stop=True)
                    hT = work.tile([128, n], F32, tag=f"hT{l % 2}",
                                   name=f"hT{l}")
                    nc.vector.tensor_scalar(hT, hps, gb[l], None, OP.add)

                    # s = [sj; si] = aw.T @ hT
                    sps = gps_misc.tile([2, n], F32, tag="misc")
                    for h0 in range(0, n, MM):
                        nc.tensor.matmul(sps[:, h0:h0 + MM], gaw[l],
                                         hT[:, h0:h0 + MM],
                                         start=True, stop=True)
                    s_sb = work.tile([2, n], F32, tag="s_sb")
                    nc.vector.tensor_copy(s_sb, sps)

                    # si broadcast [128, n] via rank-1 PE matmul (stays in PSUM)
                    sib = gps_big.tile([128, n], F32, tag="big")
                    for h0 in range(0, n, MM):
                        nc.tensor.matmul(sib[:, h0:h0 + MM], ones_row,
                                         s_sb[0:1, h0:h0 + MM],
                                         start=True, stop=True)

                    outp = gps_big.tile([128, n], F32, tag="big")
                    hcols = work.tile([128, nch, 128], F32, tag="hcols")
                    for jc in range(nch):
                        js = slice(jc * 128, (jc + 1) * 128)
                        # sj column for this chunk (+ab)
                        tp = gps_misc.tile([128, 2], F32, tag="misc",
                                           name=f"tp{l}_{jc}")
                        nc.tensor.transpose(tp, s_sb[:, js], ident[0:2, 0:2])
                        sjab = small.tile([128, 1], F32, tag="sjab")
                        nc.vector.tensor_scalar(sjab, tp[:, 1:2], gab[l],
                                                None, OP.add)
                        # att chunk
                        att = att_p.tile([128, n], F32, tag="att")
                        nc.scalar.activation(att, sib, AF.Sigmoid, bias=sjab)
                        nc.vector.tensor_tensor(att, att, adj_sb[:, jc, :],
                                                OP.mult)
                        # h columns (normal orientation) for lhsT
                        hc = gps_misc.tile([128, 128], F32, tag="misc",
                                           name=f"hc{l}_{jc}")
                        nc.tensor.transpose(hc, hT[:, js], ident)
                        nc.vector.tensor_copy(hcols[:, jc, :], hc)
                        for h0 in range(0, n, MM):
                            nc.tensor.matmul(outp[:, h0:h0 + MM],
                                             hcols[:, jc, :],
                                             att[:, h0:h0 + MM],
                                             start=(jc == 0), stop=(jc == nch - 1))
                    nxt = work.tile([128, n], F32, tag=f"hT{(l + 1) % 2}",
                                    name=f"x{l + 1}")
                    if l < 2:
                        nc.vector.tensor_scalar(nxt, outp, 0.0, None, OP.max)
                    else:
                        nc.vector.tensor_copy(nxt, outp)
                    cur = nxt
                    din = H
                hT3 = cur

            # ---- LSTM + heads phase --------------------------------------
            lo0 = const.tile([128, n], F32)
            lo1 = const.tile([128, n], F32)
            p0t4 = const.tile([128, 4 * n], F32)
            p0v = p0t4.rearrange("p (t four) -> p t four", four=4)

            zcol = const.tile([128, 1], F32)
            nc.vector.memset(zcol, 0.0)
            cst = [const.tile([128, 1], F32, name=f"c{l}") for l in range(2)]
            for l in range(2):
                nc.vector.memset(cst[l], 0.0)

            with (
                tc.tile_pool(name="lps", bufs=4, space="PSUM") as lps,
                tc.tile_pool(name="gpsg0", bufs=2, space="PSUM") as psg0,
                tc.tile_pool(name="gpsg1", bufs=2, space="PSUM") as psg1,
            ):
                psg = [psg0, psg1]

                # P0T4 = Wih0.T @ hT3 (+bsum0), interleaved [., 4t+g]
                for g in range(4):
                    gs = slice(g * 128, (g + 1) * 128)
                    for h0 in range(0, n, MM):
                        pp = lps.tile([128, MM], F32, tag="cps",
                                      name=f"p0ps{g}_{h0}")
                        nc.tensor.matmul(pp, wihT[0][:, gs],
                                         hT3[:, h0:h0 + MM],
                                         start=True, stop=True)
                        dst = p0v[:, h0:h0 + MM, g]
                        if g % 2 == 0:
                            nc.scalar.activation(dst, pp, AF.Identity,
                                                 bias=bsum[0][:, g:g + 1])
                        else:
                            nc.vector.tensor_scalar(dst, pp,
                                                    bsum[0][:, g:g + 1],
                                                    None, OP.add)

                # node head on hT3 (fills gaps during the LSTM scan)
                def head_chunk(hn, dout, src, k):
                    ts0 = k * HC
                    nblk = (dout + 127) // 128
                    m1ps = lps.tile([128, HC], F32, tag="cps",
                                    name=f"m1ps{hn}{k}")
                    nc.tensor.matmul(m1ps, hw1[hn], src[:, ts0:ts0 + HC],
                                     start=True, stop=True)
                    m1 = work.tile([128, HC], F32, tag="m1")
                    nc.vector.tensor_scalar(m1, m1ps, hb1[hn], 0.0,
                                            OP.add, OP.max)
                    for ob in range(nblk):
                        d = min(128, dout - ob * 128)
                        obs = slice(ob * 128, ob * 128 + d)
                        o2ps = lps.tile([128, HC], F32, tag="cps",
                                        name=f"o2ps{hn}{k}{ob}")
                        nc.tensor.matmul(o2ps[:d, :], hw2[hn][:, obs], m1,
                                         start=True, stop=True)
                        osb = work.tile([128, HC], F32, tag="osb")
                        nc.scalar.activation(osb[:d, :], o2ps[:d, :],
                                             AF.Identity,
                                             bias=hb2[hn][:d, ob:ob + 1])
                        tout = work.tile([128, (HC // 128) * 128], F32,
                                         tag="tout", name=f"tout{hn}{k}{ob}")
                        tov = tout.rearrange("p (tb d) -> p tb d", d=128)
                        for tb in range(HC // 128):
                            tps = lps.tile([128, 128], F32, tag="cps",
                                           name=f"tps{hn}{k}{ob}{tb}")
                            nc.tensor.transpose(
                                tps[:128, :d],
                                osb[:d, tb * 128:(tb + 1) * 128],
                                ident[:d, :d])
                            if ob % 2 == 0:
                                nc.vector.tensor_copy(tov[:, tb, :d],
                                                      tps[:128, :d])
                            else:
                                nc.scalar.copy(tov[:, tb, :d], tps[:128, :d])
                        dst = hout_d[hn][ts0:ts0 + HC, obs]
                        dstv = dst.rearrange("(tb p) d -> p tb d", p=128)
                        nc.sync.dma_start(out=dstv, in_=tov[:, :, :d])

                for k in range(nhc):
                    head_chunk("no", ND, hT3, k)

                # ---- the scan ----
                def lstm_chunk(l, k, src4v, lo):
                    """Emit CH steps of layer l for chunk k.
                    src4v: [128, n, 4] interleaved pre-gate x-part view."""
                    for tau in range(CH):
                        t = k * CH + tau
                        hprev = zcol if t == 0 else lo[:, t - 1:t]
                        ps4 = psg[l].tile([128, 4], F32, tag=f"psg{l}",
                                          name=f"ps4_{l}_{t}")
                        for g in range(4):
                            nc.tensor.matmul(
                                ps4[:, g:g + 1],
                                whhT[l][:, g * 128:(g + 1) * 128],
                                hprev, start=True, stop=True)
                        gpre = small.tile([128, 4], F32, tag=f"gpre{l}",
                                          name=f"gpre{l}_{t}")
                        nc.vector.tensor_tensor(gpre, ps4, src4v[:, t, :],
                                                OP.add)
                        s3 = small.tile([128, 3], F32, tag=f"s3_{l}",
                                        name=f"s3_{l}_{t}")
                        nc.scalar.activation(s3, gpre[:, 0:3], AF.Sigmoid)
                        tg = small.tile([128, 1], F32, tag=f"tg{l}",
                                        name=f"tg{l}_{t}")
                        nc.scalar.activation(tg, gpre[:, 3:4], AF.Tanh)
                        u = small.tile([128, 1], F32, tag=f"u{l}",
                                       name=f"u{l}_{t}")
                        nc.vector.tensor_scalar(u, tg, s3[:, 0:1], None,
                                                OP.mult)
                        # c = c*sig_f + u   (in place)
                        nc.vector.scalar_tensor_tensor(
                            cst[l], cst[l], s3[:, 1:2], u, OP.mult, OP.add)
                        tc_ = small.tile([128, 1], F32, tag=f"tc{l}",
                                         name=f"tc{l}_{t}")
                        nc.scalar.activation(tc_, cst[l], AF.Tanh)
                        nc.vector.tensor_scalar(lo[:, t:t + 1], tc_,
                                                s3[:, 2:3], None, OP.mult)

                def wih1_chunk(k):
                    ts0 = k * CH
                    for g in range(4):
                        gs = slice(g * 128, (g + 1) * 128)
                        pp = lps.tile([128, CH], F32, tag="cps",
                                      name=f"p1ps{g}_{k}")
                        nc.tensor.matmul(pp, wihT[1][:, gs],
                                         lo0[:, ts0:ts0 + CH],
                                         start=True, stop=True)
                        dst = p1v[:, ts0:ts0 + CH, g]
                        if g % 2 == 0:
                            nc.scalar.activation(dst, pp, AF.Identity,
                                                 bias=bsum[1][:, g:g + 1])
                        else:
                            nc.vector.tensor_scalar(dst, pp,
                                                    bsum[1][:, g:g + 1],
                                                    None, OP.add)

                p1t4 = const.tile([128, 4 * n], F32)
                p1v = p1t4.rearrange("p (t four) -> p t four", four=4)

                lstm_chunk(0, 0, p0v, lo0)
                wih1_chunk(0)
                for k in range(1, nlch):
                    lstm_chunk(0, k, p0v, lo0)
                    lstm_chunk(1, k - 1, p1v, lo1)
                    wih1_chunk(k)
                lstm_chunk(1, nlch - 1, p1v, lo1)

                # op/pa/sk heads on lo1
                for k in range(nhc):
                    for hn, dout in HEADS[:3]:
                        head_chunk(hn, dout, lo1, k)

    nsplit = _split_multi_waits(nc)
    if int(os.environ.get("BASSK_VERBOSE", "0")):
        print(f"build: split {nsplit} extra waits")
    return nc


# ---------------------------------------------------------------------------
def _prep_core_inputs(b, node_features, adjacency, weights, n=N):
    """Build the per-core input map (numpy) for batch element b."""
    w = weights
    inm = {
        "xT": np.ascontiguousarray(node_features[b].T),          # [64, n]
        "adjT": np.ascontiguousarray(adjacency[b].T),            # [n, n]
        "ident": np.eye(128, dtype=np.float32),
    }
    for l, pre in enumerate(("g1", "g2", "g3")):
        inm[f"g{l}_w"] = w[pre + "_W"]
        inm[f"g{l}_b"] = w[pre + "_b"].reshape(H, 1)
        aw = w[pre + "_aW"]
        # col 0 = si weights (aW[H:]), col 1 = sj weights (aW[:H]) so the
        # broadcast row (si) sits at base partition 0 of s
        inm[f"g{l}_aw"] = np.ascontiguousarray(
            np.stack([aw[H:], aw[:H]], axis=1))                  # [128, 2]
        inm[f"g{l}_ab"] = np.full((H, 1), np.float32(w[pre + "_ab"]),
                                  dtype=np.float32)
    for l in range(2):
        wih = w[f"l{l}_Wih"][_GATE_PERM]
        whh = w[f"l{l}_Whh"][_GATE_PERM]
        bs = (w[f"l{l}_bih"] + w[f"l{l}_bhh"])[_GATE_PERM]
        inm[f"l{l}_wihT"] = np.ascontiguousarray(wih.T)          # [128, 512]
        inm[f"l{l}_whhT"] = np.ascontiguousarray(whh.T)
        inm[f"l{l}_bsum"] = np.ascontiguousarray(
            bs.reshape(4, 128).T)                                # [128, 4]
    for hn, src, dout in (("op", "op", NUM_OPS), ("pa", "pa", NUM_PARAMS),
                          ("sk", "sk", SKETCH), ("no", "no", ND)):
        inm[f"{hn}_w1"] = w[src + "_W1"]
        inm[f"{hn}_b1"] = w[src + "_b1"].reshape(H, 1)
        inm[f"{hn}_w2"] = w[src + "_W2"]
        b2 = w[src + "_b2"]
        nblk = (dout + 127) // 128
        inm[f"{hn}_b2"] = np.ascontiguousarray(
            b2.reshape(nblk, min(dout, 128)).T)                  # [<=128, nblk]
    return {k: np.ascontiguousarray(np.asarray(v, dtype=np.float32))
            for k, v in inm.items()}


_CACHE = {}


def kernel(node_features, adjacency, mask, **w):
    node_features = np.asarray(node_features, dtype=np.float32)
    adjacency = np.asarray(adjacency, dtype=np.float32)
    w = {k: np.asarray(v, dtype=np.float32) for k, v in w.items()}

    if "nc" not in _CACHE:
        _CACHE["nc"] = build_program(N)
    nc = _CACHE["nc"]

    in_maps = [_prep_core_inputs(b, node_features, adjacency, w)
               for b in range(NCORES)]

    trace = bool(int(os.environ.get("BASSK_TRACE", "0")))
    res = bass_utils.run_bass_kernel_spmd(
        nc, in_maps, core_ids=list(range(NCORES)), trace=trace)
    if trace and res.exec_time_ns is not None:
        print(f"HW exec time: {res.exec_time_ns} ns")
    _CACHE["last_results"] = res

    op = np.stack([res.results[b]["op_out"] for b in range(B)])
    pa = np.stack([res.results[b]["pa_out"] for b in range(B)])
    sk = np.stack([res.results[b]["sk_out"] for b in range(B)])
    no = np.stack([res.results[b]["no_out"] for b in range(B)])
    return (op, pa, sk, no)


# revision 10
# speedup vs baseline: 1.6938x; 1.6938x over previous
"""Trainium2 Bass kernel for nn_DeepCAD (3x GNN-attention layers -> 2x LSTM -> 4 MLP heads).

Sharding: data-parallel over batch B=8 across the 8 NeuronCores (one batch
element per core); small weights replicated; host does the scatter/gather.

Per-core layout strategy:
  - GNN runs in "transposed" space: hT [H=128 partitions, N=1024 free].
  - attention: attT[j, i] = sigmoid(sj[j] + si[i] + ab) * adjT[j, i], built
    per 128-row chunk of j with sigmoid-bias = per-partition sj column and a
    PE-broadcast si row; att @ h via PE with lhsT = h (normal orientation,
    obtained by PE transposes of hT).
  - LSTM: x-parts precomputed as one big matmul into an interleaved
    [128, 4t+g] SBUF buffer; the 1024-step scan runs fully unrolled with
    gates-on-partitions [128, 4] tiles; gate col order (i, f, o, g) so one
    Sigmoid covers i,f,o and one Tanh covers g. Layer 1 is chunk-pipelined
    behind layer 0 (wavefront).
  - heads: chunked matmuls + PE transposes back to [t, dout] rows.
"""

import os
import numpy as np

import concourse.bass as bass
import concourse.tile as tile
from concourse import mybir
from concourse import bass_utils
from concourse.vector_clock import ScopedClock, VectorClock

F32 = mybir.dt.float32
FR = mybir.dt.bfloat16
AF = mybir.ActivationFunctionType
OP = mybir.AluOpType

B, N, ND, H = 8, 1024, 64, 128
NUM_OPS, NUM_PARAMS, SKETCH = 64, 256, 128
NCORES = 8

# gate permutation: torch order (i, f, g, o) -> our col order (i, f, o, g)
_GATE_PERM = np.concatenate([
    np.arange(0, 128),       # i
    np.arange(128, 256),     # f
    np.arange(384, 512),     # o
    np.arange(256, 384),     # g
])


# ---------------------------------------------------------------------------
# Workaround: this walrus encodes at most ONE sem-wait per Drain instruction.
# Tile's exit drain waits on every allocated sem at once -> split it.
def _drain_and_barrier_split(self, tick_clock, wait_clock):
    nc = self.nc
    vclock = tick_clock.global_clock
    emitted = False
    n = len(vclock)
    for p in range(n):
        t = vclock[p]
        if t <= 0:
            continue
        vc = VectorClock([0] * n)
        vc.require_at_least(p, t)
        d = nc.sync.drain()
        wait_clock.add_sem_waits(d.ins, ScopedClock({None: vc}))
        emitted = True
    if not emitted:
        nc.sync.drain()

    nc.all_engine_barrier()
    assert self.sems is not None
    popped = nc._tile_sem_poison_stack.pop()
    assert popped is self._sem_poison
    nc.clear_and_free_semaphores(list(self.sems.allocated().values()))
    nc.all_engine_barrier()


def _apply_tile_patch():
    tile.TileContext._drain_and_barrier = _drain_and_barrier_split


def _split_multi_waits(nc, limit=1):
    """This walrus encodes at most `limit` sem-waits per instruction; hoist
    extra waits onto same-engine NoOps inserted just before the instruction."""
    total = 0
    for f in nc.m.functions:
        for bb in f.blocks:
            insts = bb.instructions
            new_list = []
            for inst in insts:
                si = inst.sync_info
                if si is not None:
                    waits = list(si.on_wait)
                    if len(waits) > limit:
                        for w in waits[:-limit]:
                            nop = mybir.InstNoOp(name=f"waitsplit-{total}")
                            total += 1
                            nop.engine = inst.engine
                            nop.sync_info = mybir.SyncInfo(
                                on_wait=[w], on_update=[])
                            new_list.append(nop)
                        inst.sync_info = mybir.SyncInfo(
                            on_wait=waits[-limit:],
                            on_update=list(si.on_update))
                new_list.append(inst)
            if len(new_list) != len(insts):
                insts[:] = new_list
    return total


# ---------------------------------------------------------------------------
def build_program(n=N):
    """Build the per-core Bass program (SPMD: same program, per-core data)."""
    _apply_tile_patch()
    nc = bass.Bass("TRN2", target_bir_lowering=False, debug=False)

    nch = n // 128          # GNN j-chunks
    MM = min(512, n)        # moving-operand matmul width
    CH = min(128, n)        # LSTM emission chunk
    nlch = n // CH          # LSTM chunks
    HC = min(512, n)        # head chunk
    nhc = n // HC

    din0 = ND

    # ---- DRAM tensors -----------------------------------------------------
    def din_(name, shape):
        return nc.dram_tensor(name, list(shape), F32, kind="ExternalInput")

    xT_d = din_("xT", [din0, n])
    zeros_d = nc.dram_tensor("zeros1", [128, 1], FR, kind="ExternalInput")
    adjT_d = din_("adjT", [n, n])
    ident_d = din_("ident", [128, 128])

    gw_d, gb_d, gaw_d, gab_d = [], [], [], []
    for l in range(3):
        din = din0 if l == 0 else H
        gw_d.append(din_(f"g{l}_w", [din, H]))
        gb_d.append(din_(f"g{l}_b", [H, 1]))
        gaw_d.append(din_(f"g{l}_aw", [H, 2]))
        gab_d.append(din_(f"g{l}_ab", [H, 1]))

    wihT_d, whhT_d, bsum_d = [], [], []
    for l in range(2):
        wihT_d.append(nc.dram_tensor(f"l{l}_wihT", [H, 4 * H],
                                     F32 if l == 0 else FR,
                                     kind="ExternalInput"))
        whhT_d.append(nc.dram_tensor(f"l{l}_whhT", [H, 4 * H], FR,
                                     kind="ExternalInput"))
        bsum_d.append(din_(f"l{l}_bsum", [H, 4]))

    HEADS = [("op", NUM_OPS), ("pa", NUM_PARAMS), ("sk", SKETCH), ("no", ND)]
    hw1_d, hb1_d, hw2_d, hb2_d, hout_d = {}, {}, {}, {}, {}
    for hn, dout in HEADS:
        hw1_d[hn] = nc.dram_tensor(f"{hn}_w1", [H, H],
                                   F32 if hn == "no" else FR,
                                   kind="ExternalInput")
        hb1_d[hn] = din_(f"{hn}_b1", [H, 1])
        hw2_d[hn] = din_(f"{hn}_w2", [H, dout])
        nblk = (dout + 127) // 128
        hb2_d[hn] = din_(f"{hn}_b2", [min(dout, 128), nblk])
        hout_d[hn] = nc.dram_tensor(f"{hn}_out", [n, dout], F32,
                                    kind="ExternalOutput")

    with tile.TileContext(nc) as tc:
        with (
            tc.tile_pool(name="const", bufs=1) as const,
            tc.tile_pool(name="work", bufs=2) as work,
            tc.tile_pool(name="att_p", bufs=3) as att_p,
            tc.tile_pool(name="small", bufs=4) as small,
        ):
            # ---- load constants/weights ----------------------------------
            ident = const.tile([128, 128], F32)
            nc.sync.dma_start(out=ident, in_=ident_d[:, :])
            ones_row = const.tile([1, 128], F32)
            nc.vector.memset(ones_row, 1.0)

            adj_sb = const.tile([128, nch, n], F32)
            for jc in range(nch):
                nc.sync.dma_start(out=adj_sb[:, jc, :],
                                  in_=adjT_d[jc * 128:(jc + 1) * 128, :])

            xT_sb = const.tile([din0, n], F32)
            nc.sync.dma_start(out=xT_sb, in_=xT_d[:, :])

            gw, gb, gaw, gab = [], [], [], []
            for l in range(3):
                din = din0 if l == 0 else H
                w = const.tile([din, H], F32, name=f"gw{l}")
                nc.sync.dma_start(out=w, in_=gw_d[l][:, :])
                b = const.tile([H, 1], F32, name=f"gb{l}")
                nc.sync.dma_start(out=b, in_=gb_d[l][:, :])
                aw = const.tile([H, 2], F32, name=f"gaw{l}")
                nc.sync.dma_start(out=aw, in_=gaw_d[l][:, :])
                ab = const.tile([H, 1], F32, name=f"gab{l}")
                nc.sync.dma_start(out=ab, in_=gab_d[l][:, :])
                gw.append(w); gb.append(b); gaw.append(aw); gab.append(ab)

            wihT, whhT, bsum = [], [], []
            for l in range(2):
                a = const.tile([H, 4 * H], F32 if l == 0 else FR,
                               name=f"wihT{l}")
                nc.sync.dma_start(out=a, in_=wihT_d[l][:, :])
                b_ = const.tile([H, 4 * H], FR, name=f"whhT{l}")
                nc.sync.dma_start(out=b_, in_=whhT_d[l][:, :])
                c_ = const.tile([H, 4], F32, name=f"bsum{l}")
                nc.sync.dma_start(out=c_, in_=bsum_d[l][:, :])
                wihT.append(a); whhT.append(b_); bsum.append(c_)

            hw1, hb1, hw2, hb2 = {}, {}, {}, {}
            for hn, dout in HEADS:
                nblk = (dout + 127) // 128
                hw1[hn] = const.tile([H, H], F32 if hn == "no" else FR,
                                     name=f"hw1{hn}")
                nc.sync.dma_start(out=hw1[hn], in_=hw1_d[hn][:, :])
                hb1[hn] = const.tile([H, 1], F32, name=f"hb1{hn}")
                nc.sync.dma_start(out=hb1[hn], in_=hb1_d[hn][:, :])
                hw2[hn] = const.tile([H, dout], F32, name=f"hw2{hn}")
                nc.sync.dma_start(out=hw2[hn], in_=hw2_d[hn][:, :])
                hb2[hn] = const.tile([min(dout, 128), nblk], F32,
                                     name=f"hb2{hn}")
                nc.sync.dma_start(out=hb2[hn], in_=hb2_d[hn][:, :])

            # ---- GNN phase (own PSUM scope) ------------------------------
            cur = xT_sb
            din = din0
            hT3 = None
            with (
                tc.tile_pool(name="gps_big", bufs=2, space="PSUM") as gps_big,
                tc.tile_pool(name="gps_misc", bufs=2, space="PSUM") as gps_misc,
            ):
                for l in range(3):
                    # hT = W.T @ cur + b
                    hps = gps_big.tile([128, n], F32, tag="big")
                    for h0 in range(0, n, MM):
                        nc.tensor.matmul(hps[:, h0:h0 + MM], gw[l][:din, :],
                                         cur[:din, h0:h0 + MM],
                                         start=True, stop=True)
                    hT = work.tile([128, n], F32, tag=f"hT{l % 2}",
                                   name=f"hT{l}")
                    nc.vector.tensor_scalar(hT, hps, gb[l], None, OP.add)

                    # s = [sj; si] = aw.T @ hT
                    sps = gps_misc.tile([2, n], F32, tag="misc")
                    for h0 in range(0, n, MM):
                        nc.tensor.matmul(sps[:, h0:h0 + MM], gaw[l],
                                         hT[:, h0:h0 + MM],
                                         start=True, stop=True)
                    s_sb = work.tile([2, n], F32, tag="s_sb")
                    nc.vector.tensor_copy(s_sb, sps)

                    # si broadcast [128, n] via rank-1 PE matmul (stays in PSUM)
                    sib = gps_big.tile([128, n], F32, tag="big")
                    for h0 in range(0, n, MM):
                        nc.tensor.matmul(sib[:, h0:h0 + MM], ones_row,
                                         s_sb[0:1, h0:h0 + MM],
                                         start=True, stop=True)

                    outp = gps_big.tile([128, n], F32, tag="big")
                    hcols = work.tile([128, nch, 128], F32, tag="hcols")
                    for jc in range(nch):
                        js = slice(jc * 128, (jc + 1) * 128)
                        # sj column for this chunk (+ab)
                        tp = gps_misc.tile([128, 2], F32, tag="misc",
                                           name=f"tp{l}_{jc}")
                        nc.tensor.transpose(tp, s_sb[:, js], ident[0:2, 0:2])
                        sjab = small.tile([128, 1], F32, tag="sjab")
                        nc.vector.tensor_scalar(sjab, tp[:, 1:2], gab[l],
                                                None, OP.add)
                        # att chunk
                        att = att_p.tile([128, n], F32, tag="att")
                        nc.scalar.activation(att, sib, AF.Sigmoid, bias=sjab)
                        nc.vector.tensor_tensor(att, att, adj_sb[:, jc, :],
                                                OP.mult)
                        # h columns (normal orientation) for lhsT
                        hc = gps_misc.tile([128, 128], F32, tag="misc",
                                           name=f"hc{l}_{jc}")
                        nc.tensor.transpose(hc, hT[:, js], ident)
                        nc.vector.tensor_copy(hcols[:, jc, :], hc)
                        for h0 in range(0, n, MM):
                            nc.tensor.matmul(outp[:, h0:h0 + MM],
                                             hcols[:, jc, :],
                                             att[:, h0:h0 + MM],
                                             start=(jc == 0), stop=(jc == nch - 1))
                    nxt = work.tile([128, n], F32, tag=f"hT{(l + 1) % 2}",
                                    name=f"x{l + 1}")
                    if l < 2:
                        nc.vector.tensor_scalar(nxt, outp, 0.0, None, OP.max)
                    else:
                        nc.vector.tensor_copy(nxt, outp)
                    cur = nxt
                    din = H
                hT3 = cur

            # ---- LSTM + heads phase --------------------------------------
            lo0 = const.tile([128, n], FR)
            lo1 = const.tile([128, n], FR)
            p0t4 = const.tile([128, 4 * n], F32)
            p0v = p0t4.rearrange("p (t four) -> p t four", four=4)

            zcol = const.tile([128, 1], FR)
            nc.sync.dma_start(out=zcol, in_=zeros_d[:, :])
            cst = [const.tile([128, 1], F32, name=f"c{l}") for l in range(2)]
            for l in range(2):
                nc.vector.memset(cst[l], 0.0)

            with (
                tc.tile_pool(name="lps", bufs=4, space="PSUM") as lps,
                tc.tile_pool(name="gpsg0", bufs=2, space="PSUM") as psg0,
                tc.tile_pool(name="gpsg1", bufs=2, space="PSUM") as psg1,
            ):
                psg = [psg0, psg1]

                # P0T4 = Wih0.T @ hT3 (+bsum0), interleaved [., 4t+g]
                for g in range(4):
                    gs = slice(g * 128, (g + 1) * 128)
                    for h0 in range(0, n, MM):
                        pp = lps.tile([128, MM], F32, tag="cps",
                                      name=f"p0ps{g}_{h0}")
                        nc.tensor.matmul(pp, wihT[0][:, gs],
                                         hT3[:, h0:h0 + MM],
                                         start=True, stop=True)
                        dst = p0v[:, h0:h0 + MM, g]
                        if g % 2 == 0:
                            nc.scalar.activation(dst, pp, AF.Identity,
                                                 bias=bsum[0][:, g:g + 1])
                        else:
                            nc.vector.tensor_scalar(dst, pp,
                                                    bsum[0][:, g:g + 1],
                                                    None, OP.add)

                # node head on hT3 (fills gaps during the LSTM scan)
                def head_chunk(hn, dout, src, k):
                    ts0 = k * HC
                    nblk = (dout + 127) // 128
                    m1ps = lps.tile([128, HC], F32, tag="cps",
                                    name=f"m1ps{hn}{k}")
                    nc.tensor.matmul(m1ps, hw1[hn], src[:, ts0:ts0 + HC],
                                     start=True, stop=True)
                    m1 = work.tile([128, HC], F32, tag="m1")
                    nc.vector.tensor_scalar(m1, m1ps, hb1[hn], 0.0,
                                            OP.add, OP.max)
                    for ob in range(nblk):
                        d = min(128, dout - ob * 128)
                        obs = slice(ob * 128, ob * 128 + d)
                        o2ps = lps.tile([128, HC], F32, tag="cps",
                                        name=f"o2ps{hn}{k}{ob}")
                        nc.tensor.matmul(o2ps[:d, :], hw2[hn][:, obs], m1,
                                         start=True, stop=True)
                        osb = work.tile([128, HC], F32, tag="osb")
                        nc.scalar.activation(osb[:d, :], o2ps[:d, :],
                                             AF.Identity,
                                             bias=hb2[hn][:d, ob:ob + 1])
                        tout = work.tile([128, (HC // 128) * 128], F32,
                                         tag="tout", name=f"tout{hn}{k}{ob}")
                        tov = tout.rearrange("p (tb d) -> p tb d", d=128)
                        for tb in range(HC // 128):
                            tps = lps.tile([128, 128], F32, tag="cps",
                                           name=f"tps{hn}{k}{ob}{tb}")
                            nc.tensor.transpose(
                                tps[:128, :d],
                                osb[:d, tb * 128:(tb + 1) * 128],
                                ident[:d, :d])
                            if ob % 2 == 0:
                                nc.vector.tensor_copy(tov[:, tb, :d],
                                                      tps[:128, :d])
                            else:
                                nc.scalar.copy(tov[:, tb, :d], tps[:128, :d])
                        dst = hout_d[hn][ts0:ts0 + HC, obs]
                        dstv = dst.rearrange("(tb p) d -> p tb d", p=128)
                        nc.sync.dma_start(out=dstv, in_=tov[:, :, :d])

                for k in range(nhc):
                    head_chunk("no", ND, hT3, k)

                # ---- the scan ----
                def lstm_chunk(l, k, src4v, lo):
                    """Emit CH steps of layer l for chunk k.
                    src4v: [128, n, 4] interleaved pre-gate x-part view."""
                    for tau in range(CH):
                        t = k * CH + tau
                        hprev = zcol if t == 0 else lo[:, t - 1:t]
                        ps4 = psg[l].tile([128, 4], F32, tag=f"psg{l}",
                                          name=f"ps4_{l}_{t}")
                        for g in range(4):
                            nc.tensor.matmul(
                                ps4[:, g:g + 1],
                                whhT[l][:, g * 128:(g + 1) * 128],
                                hprev, start=True, stop=True)
                        gpre = small.tile([128, 4], F32, tag=f"gpre{l}",
                                          name=f"gpre{l}_{t}")
                        nc.vector.tensor_tensor(gpre, ps4, src4v[:, t, :],
                                                OP.add)
                        s3 = small.tile([128, 3], F32, tag=f"s3_{l}",
                                        name=f"s3_{l}_{t}")
                        nc.scalar.activation(s3, gpre[:, 0:3], AF.Sigmoid)
                        tg = small.tile([128, 1], F32, tag=f"tg{l}",
                                        name=f"tg{l}_{t}")
                        nc.scalar.activation(tg, gpre[:, 3:4], AF.Tanh)
                        u = small.tile([128, 1], F32, tag=f"u{l}",
                                       name=f"u{l}_{t}")
                        nc.gpsimd.tensor_scalar(u, tg, s3[:, 0:1], None,
                                                OP.mult)
                        # c = c*sig_f + u   (in place)
                        nc.vector.scalar_tensor_tensor(
                            cst[l], cst[l], s3[:, 1:2], u, OP.mult, OP.add)
                        tc_ = small.tile([128, 1], F32, tag=f"tc{l}",
                                         name=f"tc{l}_{t}")
                        nc.scalar.activation(tc_, cst[l], AF.Tanh)
                        nc.gpsimd.tensor_scalar(lo[:, t:t + 1], tc_,
                                                s3[:, 2:3], None, OP.mult)

                def wih1_chunk(k):
                    ts0 = k * CH
                    for g in range(4):
                        gs = slice(g * 128, (g + 1) * 128)
                        pp = lps.tile([128, CH], F32, tag="cps",
                                      name=f"p1ps{g}_{k}")
                        nc.tensor.matmul(pp, wihT[1][:, gs],
                                         lo0[:, ts0:ts0 + CH],
                                         start=True, stop=True)
                        dst = p1v[:, ts0:ts0 + CH, g]
                        if g % 2 == 0:
                            nc.scalar.activation(dst, pp, AF.Identity,
                                                 bias=bsum[1][:, g:g + 1])
                        else:
                            nc.vector.tensor_scalar(dst, pp,
                                                    bsum[1][:, g:g + 1],
                                                    None, OP.add)

                p1t4 = const.tile([128, 4 * n], F32)
                p1v = p1t4.rearrange("p (t four) -> p t four", four=4)

                lstm_chunk(0, 0, p0v, lo0)
                wih1_chunk(0)
                for k in range(1, nlch):
                    lstm_chunk(0, k, p0v, lo0)
                    lstm_chunk(1, k - 1, p1v, lo1)
                    wih1_chunk(k)
                lstm_chunk(1, nlch - 1, p1v, lo1)

                # op/pa/sk heads on lo1
                for k in range(nhc):
                    for hn, dout in HEADS[:3]:
                        head_chunk(hn, dout, lo1, k)

    nsplit = _split_multi_waits(nc)
    if int(os.environ.get("BASSK_VERBOSE", "0")):
        print(f"build: split {nsplit} extra waits")
    return nc


# ---------------------------------------------------------------------------
def _prep_core_inputs(b, node_features, adjacency, weights, n=N):
    """Build the per-core input map (numpy) for batch element b."""
    w = weights
    import ml_dtypes
    bf16 = ml_dtypes.bfloat16
    inm = {
        "zeros1": np.zeros((128, 1), bf16),
        "xT": np.ascontiguousarray(node_features[b].T),          # [64, n]
        "adjT": np.ascontiguousarray(adjacency[b].T),            # [n, n]
        "ident": np.eye(128, dtype=np.float32),
    }
    for l, pre in enumerate(("g1", "g2", "g3")):
        inm[f"g{l}_w"] = w[pre + "_W"]
        inm[f"g{l}_b"] = w[pre + "_b"].reshape(H, 1)
        aw = w[pre + "_aW"]
        # col 0 = si weights (aW[H:]), col 1 = sj weights (aW[:H]) so the
        # broadcast row (si) sits at base partition 0 of s
        inm[f"g{l}_aw"] = np.ascontiguousarray(
            np.stack([aw[H:], aw[:H]], axis=1))                  # [128, 2]
        inm[f"g{l}_ab"] = np.full((H, 1), np.float32(w[pre + "_ab"]),
                                  dtype=np.float32)
    for l in range(2):
        wih = w[f"l{l}_Wih"][_GATE_PERM]
        whh = w[f"l{l}_Whh"][_GATE_PERM]
        bs = (w[f"l{l}_bih"] + w[f"l{l}_bhh"])[_GATE_PERM]
        wihT = np.ascontiguousarray(wih.T)                       # [128, 512]
        inm[f"l{l}_wihT"] = wihT if l == 0 else wihT.astype(bf16)
        inm[f"l{l}_whhT"] = np.ascontiguousarray(whh.T).astype(bf16)
        inm[f"l{l}_bsum"] = np.ascontiguousarray(
            bs.reshape(4, 128).T)                                # [128, 4]
    for hn, src, dout in (("op", "op", NUM_OPS), ("pa", "pa", NUM_PARAMS),
                          ("sk", "sk", SKETCH), ("no", "no", ND)):
        w1 = w[src + "_W1"]
        inm[f"{hn}_w1"] = w1 if hn == "no" else w1.astype(bf16)
        inm[f"{hn}_b1"] = w[src + "_b1"].reshape(H, 1)
        inm[f"{hn}_w2"] = w[src + "_W2"]
        b2 = w[src + "_b2"]
        nblk = (dout + 127) // 128
        inm[f"{hn}_b2"] = np.ascontiguousarray(
            b2.reshape(nblk, min(dout, 128)).T)                  # [<=128, nblk]
    return {k: (np.ascontiguousarray(v) if v.dtype == bf16 else
                np.ascontiguousarray(np.asarray(v, dtype=np.float32)))
            for k, v in inm.items()}


_CACHE = {}


def kernel(node_features, adjacency, mask, **w):
    node_features = np.asarray(node_features, dtype=np.float32)
    adjacency = np.asarray(adjacency, dtype=np.float32)
    w = {k: np.asarray(v, dtype=np.float32) for k, v in w.items()}

    if "nc" not in _CACHE:
        _CACHE["nc"] = build_program(N)
    nc = _CACHE["nc"]

    in_maps = [_prep_core_inputs(b, node_features, adjacency, w)
               for b in range(NCORES)]

    trace = bool(int(os.environ.get("BASSK_TRACE", "0")))
    res = bass_utils.run_bass_kernel_spmd(
        nc, in_maps, core_ids=list(range(NCORES)), trace=trace)
    if trace and res.exec_time_ns is not None:
        print(f"HW exec time: {res.exec_time_ns} ns")
    _CACHE["last_results"] = res

    op = np.stack([res.results[b]["op_out"] for b in range(B)])
    pa = np.stack([res.results[b]["pa_out"] for b in range(B)])
    sk = np.stack([res.results[b]["sk_out"] for b in range(B)])
    no = np.stack([res.results[b]["no_out"] for b in range(B)])
    return (op, pa, sk, no)


# revision 12
# speedup vs baseline: 2.1714x; 1.2820x over previous
"""Trainium2 Bass kernel for nn_DeepCAD (3x GNN-attention layers -> 2x LSTM -> 4 MLP heads).

Sharding: data-parallel over batch B=8 across the 8 NeuronCores (one batch
element per core); small weights replicated; host does the scatter/gather.

Per-core layout strategy:
  - GNN runs in "transposed" space: hT [H=128 partitions, N=1024 free].
  - attention: attT[j, i] = sigmoid(sj[j] + si[i] + ab) * adjT[j, i], built
    per 128-row chunk of j with sigmoid-bias = per-partition sj column and a
    PE-broadcast si row; att @ h via PE with lhsT = h (normal orientation,
    obtained by PE transposes of hT).
  - LSTM: x-parts precomputed as one big matmul into an interleaved
    [128, 4t+g] SBUF buffer; the 1024-step scan runs fully unrolled with
    gates-on-partitions [128, 4] tiles; gate col order (i, f, o, g) so one
    Sigmoid covers i,f,o and one Tanh covers g. Layer 1 is chunk-pipelined
    behind layer 0 (wavefront).
  - heads: chunked matmuls + PE transposes back to [t, dout] rows.
"""

import os
import numpy as np

import concourse.bass as bass
import concourse.tile as tile
from concourse import mybir
from concourse import bass_utils
from concourse.vector_clock import ScopedClock, VectorClock

F32 = mybir.dt.float32
FR = mybir.dt.bfloat16
AF = mybir.ActivationFunctionType
OP = mybir.AluOpType

B, N, ND, H = 8, 1024, 64, 128
NUM_OPS, NUM_PARAMS, SKETCH = 64, 256, 128
NCORES = 8

# gate permutation: torch order (i, f, g, o) -> our col order (i, f, o, g)
_GATE_PERM = np.concatenate([
    np.arange(0, 128),       # i
    np.arange(128, 256),     # f
    np.arange(384, 512),     # o
    np.arange(256, 384),     # g
])


# ---------------------------------------------------------------------------
# Workaround: this walrus encodes at most ONE sem-wait per Drain instruction.
# Tile's exit drain waits on every allocated sem at once -> split it.
def _drain_and_barrier_split(self, tick_clock, wait_clock):
    nc = self.nc
    vclock = tick_clock.global_clock
    emitted = False
    n = len(vclock)
    for p in range(n):
        t = vclock[p]
        if t <= 0:
            continue
        vc = VectorClock([0] * n)
        vc.require_at_least(p, t)
        d = nc.sync.drain()
        wait_clock.add_sem_waits(d.ins, ScopedClock({None: vc}))
        emitted = True
    if not emitted:
        nc.sync.drain()

    nc.all_engine_barrier()
    assert self.sems is not None
    popped = nc._tile_sem_poison_stack.pop()
    assert popped is self._sem_poison
    nc.clear_and_free_semaphores(list(self.sems.allocated().values()))
    nc.all_engine_barrier()


def _apply_tile_patch():
    tile.TileContext._drain_and_barrier = _drain_and_barrier_split


def _split_multi_waits(nc, limit=1):
    """This walrus encodes at most `limit` sem-waits per instruction; hoist
    extra waits onto same-engine NoOps inserted just before the instruction."""
    total = 0
    for f in nc.m.functions:
        for bb in f.blocks:
            insts = bb.instructions
            new_list = []
            for inst in insts:
                si = inst.sync_info
                if si is not None:
                    waits = list(si.on_wait)
                    if len(waits) > limit:
                        for w in waits[:-limit]:
                            nop = mybir.InstNoOp(name=f"waitsplit-{total}")
                            total += 1
                            nop.engine = inst.engine
                            nop.sync_info = mybir.SyncInfo(
                                on_wait=[w], on_update=[])
                            new_list.append(nop)
                        inst.sync_info = mybir.SyncInfo(
                            on_wait=waits[-limit:],
                            on_update=list(si.on_update))
                new_list.append(inst)
            if len(new_list) != len(insts):
                insts[:] = new_list
    return total


# ---------------------------------------------------------------------------
def build_program(n=N):
    """Build the per-core Bass program (SPMD: same program, per-core data)."""
    _apply_tile_patch()
    nc = bass.Bass("TRN2", target_bir_lowering=False, debug=False)

    nch = n // 128          # GNN j-chunks
    MM = min(512, n)        # moving-operand matmul width
    CH = min(128, n)        # LSTM emission chunk
    nlch = n // CH          # LSTM chunks
    HC = min(512, n)        # head chunk
    nhc = n // HC

    din0 = ND

    # ---- DRAM tensors -----------------------------------------------------
    def din_(name, shape):
        return nc.dram_tensor(name, list(shape), F32, kind="ExternalInput")

    xT_d = din_("xT", [din0, n])
    zeros_d = nc.dram_tensor("zeros1", [128, 1], FR, kind="ExternalInput")
    adjT_d = din_("adjT", [n, n])
    ident_d = din_("ident", [128, 128])

    gw_d, gb_d, gaw_d, gab_d = [], [], [], []
    for l in range(3):
        din = din0 if l == 0 else H
        gw_d.append(din_(f"g{l}_w", [din, H]))
        gb_d.append(din_(f"g{l}_b", [H, 1]))
        gaw_d.append(din_(f"g{l}_aw", [H, 2]))
        gab_d.append(din_(f"g{l}_ab", [H, 1]))

    wihT_d, whhT_d, bsum_d = [], [], []
    for l in range(2):
        wihT_d.append(nc.dram_tensor(f"l{l}_wihT", [H, 4 * H],
                                     F32 if l == 0 else FR,
                                     kind="ExternalInput"))
        whhT_d.append(nc.dram_tensor(f"l{l}_whhT", [H, 4 * H], FR,
                                     kind="ExternalInput"))
        bsum_d.append(din_(f"l{l}_bsum", [H, 4]))

    HEADS = [("op", NUM_OPS), ("pa", NUM_PARAMS), ("sk", SKETCH), ("no", ND)]
    hw1_d, hb1_d, hw2_d, hb2_d, hout_d = {}, {}, {}, {}, {}
    for hn, dout in HEADS:
        hw1_d[hn] = nc.dram_tensor(f"{hn}_w1", [H, H],
                                   F32 if hn == "no" else FR,
                                   kind="ExternalInput")
        hb1_d[hn] = din_(f"{hn}_b1", [H, 1])
        hw2_d[hn] = din_(f"{hn}_w2", [H, dout])
        nblk = (dout + 127) // 128
        hb2_d[hn] = din_(f"{hn}_b2", [min(dout, 128), nblk])
        hout_d[hn] = nc.dram_tensor(f"{hn}_out", [n, dout], F32,
                                    kind="ExternalOutput")

    with tile.TileContext(nc) as tc:
        with (
            tc.tile_pool(name="const", bufs=1) as const,
            tc.tile_pool(name="work", bufs=2) as work,
            tc.tile_pool(name="att_p", bufs=3) as att_p,
            tc.tile_pool(name="small", bufs=4) as small,
        ):
            # ---- load constants/weights ----------------------------------
            ident = const.tile([128, 128], F32)
            nc.sync.dma_start(out=ident, in_=ident_d[:, :])
            ones_row = const.tile([1, 128], F32)
            nc.vector.memset(ones_row, 1.0)

            adj_sb = const.tile([128, nch, n], F32)
            for jc in range(nch):
                nc.sync.dma_start(out=adj_sb[:, jc, :],
                                  in_=adjT_d[jc * 128:(jc + 1) * 128, :])

            xT_sb = const.tile([din0, n], F32)
            nc.sync.dma_start(out=xT_sb, in_=xT_d[:, :])

            gw, gb, gaw, gab = [], [], [], []
            for l in range(3):
                din = din0 if l == 0 else H
                w = const.tile([din, H], F32, name=f"gw{l}")
                nc.sync.dma_start(out=w, in_=gw_d[l][:, :])
                b = const.tile([H, 1], F32, name=f"gb{l}")
                nc.sync.dma_start(out=b, in_=gb_d[l][:, :])
                aw = const.tile([H, 2], F32, name=f"gaw{l}")
                nc.sync.dma_start(out=aw, in_=gaw_d[l][:, :])
                ab = const.tile([H, 1], F32, name=f"gab{l}")
                nc.sync.dma_start(out=ab, in_=gab_d[l][:, :])
                gw.append(w); gb.append(b); gaw.append(aw); gab.append(ab)

            wihT, whhT, bsum = [], [], []
            for l in range(2):
                a = const.tile([H, 4 * H], F32 if l == 0 else FR,
                               name=f"wihT{l}")
                nc.sync.dma_start(out=a, in_=wihT_d[l][:, :])
                b_ = const.tile([H, 4 * H], FR, name=f"whhT{l}")
                nc.sync.dma_start(out=b_, in_=whhT_d[l][:, :])
                c_ = const.tile([H, 4], F32, name=f"bsum{l}")
                nc.sync.dma_start(out=c_, in_=bsum_d[l][:, :])
                wihT.append(a); whhT.append(b_); bsum.append(c_)

            hw1, hb1, hw2, hb2 = {}, {}, {}, {}
            for hn, dout in HEADS:
                nblk = (dout + 127) // 128
                hw1[hn] = const.tile([H, H], F32 if hn == "no" else FR,
                                     name=f"hw1{hn}")
                nc.sync.dma_start(out=hw1[hn], in_=hw1_d[hn][:, :])
                hb1[hn] = const.tile([H, 1], F32, name=f"hb1{hn}")
                nc.sync.dma_start(out=hb1[hn], in_=hb1_d[hn][:, :])
                hw2[hn] = const.tile([H, dout], F32, name=f"hw2{hn}")
                nc.sync.dma_start(out=hw2[hn], in_=hw2_d[hn][:, :])
                hb2[hn] = const.tile([min(dout, 128), nblk], F32,
                                     name=f"hb2{hn}")
                nc.sync.dma_start(out=hb2[hn], in_=hb2_d[hn][:, :])

            # ---- GNN phase (own PSUM scope) ------------------------------
            cur = xT_sb
            din = din0
            hT3 = None
            with (
                tc.tile_pool(name="gps_big", bufs=2, space="PSUM") as gps_big,
                tc.tile_pool(name="gps_misc", bufs=2, space="PSUM") as gps_misc,
            ):
                for l in range(3):
                    # hT = W.T @ cur + b
                    hps = gps_big.tile([128, n], F32, tag="big")
                    for h0 in range(0, n, MM):
                        nc.tensor.matmul(hps[:, h0:h0 + MM], gw[l][:din, :],
                                         cur[:din, h0:h0 + MM],
                                         start=True, stop=True)
                    hT = work.tile([128, n], F32, tag=f"hT{l % 2}",
                                   name=f"hT{l}")
                    nc.vector.tensor_scalar(hT, hps, gb[l], None, OP.add)

                    # s = [sj; si] = aw.T @ hT
                    sps = gps_misc.tile([2, n], F32, tag="misc")
                    for h0 in range(0, n, MM):
                        nc.tensor.matmul(sps[:, h0:h0 + MM], gaw[l],
                                         hT[:, h0:h0 + MM],
                                         start=True, stop=True)
                    s_sb = work.tile([2, n], F32, tag="s_sb")
                    nc.vector.tensor_copy(s_sb, sps)

                    # si broadcast [128, n] via rank-1 PE matmul (stays in PSUM)
                    sib = gps_big.tile([128, n], F32, tag="big")
                    for h0 in range(0, n, MM):
                        nc.tensor.matmul(sib[:, h0:h0 + MM], ones_row,
                                         s_sb[0:1, h0:h0 + MM],
                                         start=True, stop=True)

                    outp = gps_big.tile([128, n], F32, tag="big")
                    hcols = work.tile([128, nch, 128], F32, tag="hcols")
                    for jc in range(nch):
                        js = slice(jc * 128, (jc + 1) * 128)
                        # sj column for this chunk (+ab)
                        tp = gps_misc.tile([128, 2], F32, tag="misc",
                                           name=f"tp{l}_{jc}")
                        nc.tensor.transpose(tp, s_sb[:, js], ident[0:2, 0:2])
                        sjab = small.tile([128, 1], F32, tag="sjab")
                        nc.vector.tensor_scalar(sjab, tp[:, 1:2], gab[l],
                                                None, OP.add)
                        # att chunk
                        att = att_p.tile([128, n], F32, tag="att")
                        nc.scalar.activation(att, sib, AF.Sigmoid, bias=sjab)
                        nc.vector.tensor_tensor(att, att, adj_sb[:, jc, :],
                                                OP.mult)
                        # h columns (normal orientation) for lhsT
                        hc = gps_misc.tile([128, 128], F32, tag="misc",
                                           name=f"hc{l}_{jc}")
                        nc.tensor.transpose(hc, hT[:, js], ident)
                        nc.vector.tensor_copy(hcols[:, jc, :], hc)
                        for h0 in range(0, n, MM):
                            nc.tensor.matmul(outp[:, h0:h0 + MM],
                                             hcols[:, jc, :],
                                             att[:, h0:h0 + MM],
                                             start=(jc == 0), stop=(jc == nch - 1))
                    nxt = work.tile([128, n], F32, tag=f"hT{(l + 1) % 2}",
                                    name=f"x{l + 1}")
                    if l < 2:
                        nc.vector.tensor_scalar(nxt, outp, 0.0, None, OP.max)
                    else:
                        nc.vector.tensor_copy(nxt, outp)
                    cur = nxt
                    din = H
                hT3 = cur

            # ---- LSTM + heads phase --------------------------------------
            lo0 = const.tile([128, n], FR)
            lo1 = const.tile([128, n], FR)
            p0t4 = const.tile([128, 4 * n], F32)
            p0v = p0t4.rearrange("p (t four) -> p t four", four=4)

            zcol = const.tile([128, 1], FR)
            nc.sync.dma_start(out=zcol, in_=zeros_d[:, :])
            cst = [const.tile([128, 1], F32, name=f"c{l}") for l in range(2)]
            for l in range(2):
                nc.vector.memset(cst[l], 0.0)

            with (
                tc.tile_pool(name="lps", bufs=4, space="PSUM") as lps,
                tc.tile_pool(name="gpsg0", bufs=2, space="PSUM") as psg0,
                tc.tile_pool(name="gpsg1", bufs=2, space="PSUM") as psg1,
            ):
                psg = [psg0, psg1]

                # P0T4 = Wih0.T @ hT3 (+bsum0), interleaved [., 4t+g]
                for g in range(4):
                    gs = slice(g * 128, (g + 1) * 128)
                    for h0 in range(0, n, MM):
                        pp = lps.tile([128, MM], F32, tag="cps",
                                      name=f"p0ps{g}_{h0}")
                        nc.tensor.matmul(pp, wihT[0][:, gs],
                                         hT3[:, h0:h0 + MM],
                                         start=True, stop=True)
                        dst = p0v[:, h0:h0 + MM, g]
                        if g % 2 == 0:
                            nc.scalar.activation(dst, pp, AF.Identity,
                                                 bias=bsum[0][:, g:g + 1])
                        else:
                            nc.vector.tensor_scalar(dst, pp,
                                                    bsum[0][:, g:g + 1],
                                                    None, OP.add)

                # node head on hT3 (fills gaps during the LSTM scan)
                def head_chunk(hn, dout, src, k):
                    ts0 = k * HC
                    nblk = (dout + 127) // 128
                    m1ps = lps.tile([128, HC], F32, tag="cps",
                                    name=f"m1ps{hn}{k}")
                    nc.tensor.matmul(m1ps, hw1[hn], src[:, ts0:ts0 + HC],
                                     start=True, stop=True)
                    m1 = work.tile([128, HC], F32, tag="m1")
                    nc.vector.tensor_scalar(m1, m1ps, hb1[hn], 0.0,
                                            OP.add, OP.max)
                    for ob in range(nblk):
                        d = min(128, dout - ob * 128)
                        obs = slice(ob * 128, ob * 128 + d)
                        o2ps = lps.tile([128, HC], F32, tag="cps",
                                        name=f"o2ps{hn}{k}{ob}")
                        nc.tensor.matmul(o2ps[:d, :], hw2[hn][:, obs], m1,
                                         start=True, stop=True)
                        osb = work.tile([128, HC], F32, tag="osb")
                        nc.scalar.activation(osb[:d, :], o2ps[:d, :],
                                             AF.Identity,
                                             bias=hb2[hn][:d, ob:ob + 1])
                        tout = work.tile([128, (HC // 128) * 128], F32,
                                         tag="tout", name=f"tout{hn}{k}{ob}")
                        tov = tout.rearrange("p (tb d) -> p tb d", d=128)
                        for tb in range(HC // 128):
                            tps = lps.tile([128, 128], F32, tag="cps",
                                           name=f"tps{hn}{k}{ob}{tb}")
                            nc.tensor.transpose(
                                tps[:128, :d],
                                osb[:d, tb * 128:(tb + 1) * 128],
                                ident[:d, :d])
                            if ob % 2 == 0:
                                nc.vector.tensor_copy(tov[:, tb, :d],
                                                      tps[:128, :d])
                            else:
                                nc.scalar.copy(tov[:, tb, :d], tps[:128, :d])
                        dst = hout_d[hn][ts0:ts0 + HC, obs]
                        dstv = dst.rearrange("(tb p) d -> p tb d", p=128)
                        nc.sync.dma_start(out=dstv, in_=tov[:, :, :d])

                for k in range(nhc):
                    head_chunk("no", ND, hT3, k)

                # ---- the scan ----
                NWARM = int(os.environ.get("BASSK_WARM", "0"))
                warm_ps = [None]

                def emit_warm(tag):
                    for i in range(NWARM):
                        wp = psg[0].tile([128, 512], F32, tag="warm",
                                         bufs=1, name=f"warm{tag}_{i}")
                        nc.tensor.matmul(wp[:1, :], ones_row[:1, :1],
                                         adj_sb[0:1, 0, 0:512],
                                         start=True, stop=True)

                # per-(layer, t) front half: gate matmuls + x-part add +
                # one batched sigmoid (tanh(g) comes from the doubled g-row
                # trick: tanh(x) = 2*sigmoid(2x) - 1, with the 2x folded
                # into the host-side weights).
                s4_live = [None, None]

                def emit_h1(l, t, src4v, lo):
                    hprev = zcol if t == 0 else lo[:, t - 1:t]
                    ps4 = psg[l].tile([128, 4], F32, tag=f"psg{l}",
                                      name=f"ps4_{l}_{t}")
                    for g in range(4):
                        nc.tensor.matmul(
                            ps4[:, g:g + 1],
                            whhT[l][:, g * 128:(g + 1) * 128],
                            hprev, start=True, stop=True)
                    emit_warm(f"{l}_{t}")
                    gpre = small.tile([128, 4], F32, tag=f"gpre{l}",
                                      name=f"gpre{l}_{t}")
                    nc.vector.tensor_tensor(gpre, ps4, src4v[:, t, :],
                                            OP.add)
                    s4 = small.tile([128, 4], F32, tag=f"s4_{l}",
                                    name=f"s4_{l}_{t}")
                    nc.scalar.activation(s4, gpre, AF.Sigmoid)
                    s4_live[l] = s4

                # back half: state update + output
                def emit_h2(l, t, lo):
                    s4 = s4_live[l]
                    u1 = small.tile([128, 1], F32, tag=f"u1{l}",
                                    name=f"u1{l}_{t}")
                    # u1 = (sg~ * 2) * si
                    nc.vector.scalar_tensor_tensor(
                        u1, s4[:, 3:4], 2.0, s4[:, 0:1], OP.mult, OP.mult)
                    u = small.tile([128, 1], F32, tag=f"u{l}",
                                   name=f"u{l}_{t}")
                    nc.vector.tensor_tensor(u, u1, s4[:, 0:1], OP.subtract)
                    # c = c*sig_f + u   (in place)
                    nc.vector.scalar_tensor_tensor(
                        cst[l], cst[l], s4[:, 1:2], u, OP.mult, OP.add)
                    tc_ = small.tile([128, 1], F32, tag=f"tc{l}",
                                     name=f"tc{l}_{t}")
                    nc.scalar.activation(tc_, cst[l], AF.Tanh)
                    nc.vector.tensor_scalar(lo[:, t:t + 1], tc_,
                                            s4[:, 2:3], None, OP.mult)

                def lstm_chunk(l, k, src4v, lo):
                    for tau in range(CH):
                        t = k * CH + tau
                        emit_h1(l, t, src4v, lo)
                        emit_h2(l, t, lo)

                def wih1_chunk(k):
                    ts0 = k * CH
                    for g in range(4):
                        gs = slice(g * 128, (g + 1) * 128)
                        pp = lps.tile([128, CH], F32, tag="cps",
                                      name=f"p1ps{g}_{k}")
                        nc.tensor.matmul(pp, wihT[1][:, gs],
                                         lo0[:, ts0:ts0 + CH],
                                         start=True, stop=True)
                        dst = p1v[:, ts0:ts0 + CH, g]
                        if g % 2 == 0:
                            nc.scalar.activation(dst, pp, AF.Identity,
                                                 bias=bsum[1][:, g:g + 1])
                        else:
                            nc.vector.tensor_scalar(dst, pp,
                                                    bsum[1][:, g:g + 1],
                                                    None, OP.add)

                p1t4 = const.tile([128, 4 * n], F32)
                p1v = p1t4.rearrange("p (t four) -> p t four", four=4)

                lstm_chunk(0, 0, p0v, lo0)
                wih1_chunk(0)
                for k in range(1, nlch):
                    a = k * CH
                    b = (k - 1) * CH
                    for tau in range(CH):
                        emit_h1(0, a + tau, p0v, lo0)
                        if b + tau - 1 >= 0:
                            emit_h2(1, b + tau - 1, lo1)
                        emit_h2(0, a + tau, lo0)
                        emit_h1(1, b + tau, p1v, lo1)
                    wih1_chunk(k)
                # epilogue: l1's last chunk
                b = (nlch - 1) * CH
                for tau in range(CH):
                    if b + tau - 1 >= 0:
                        emit_h2(1, b + tau - 1, lo1)
                    emit_h1(1, b + tau, p1v, lo1)
                emit_h2(1, n - 1, lo1)

                # op/pa/sk heads on lo1
                for k in range(nhc):
                    for hn, dout in HEADS[:3]:
                        head_chunk(hn, dout, lo1, k)

    nsplit = _split_multi_waits(nc)
    if int(os.environ.get("BASSK_VERBOSE", "0")):
        print(f"build: split {nsplit} extra waits")
    return nc


# ---------------------------------------------------------------------------
def _prep_core_inputs(b, node_features, adjacency, weights, n=N):
    """Build the per-core input map (numpy) for batch element b."""
    w = weights
    import ml_dtypes
    bf16 = ml_dtypes.bfloat16
    inm = {
        "zeros1": np.zeros((128, 1), bf16),
        "xT": np.ascontiguousarray(node_features[b].T),          # [64, n]
        "adjT": np.ascontiguousarray(adjacency[b].T),            # [n, n]
        "ident": np.eye(128, dtype=np.float32),
    }
    for l, pre in enumerate(("g1", "g2", "g3")):
        inm[f"g{l}_w"] = w[pre + "_W"]
        inm[f"g{l}_b"] = w[pre + "_b"].reshape(H, 1)
        aw = w[pre + "_aW"]
        # col 0 = si weights (aW[H:]), col 1 = sj weights (aW[:H]) so the
        # broadcast row (si) sits at base partition 0 of s
        inm[f"g{l}_aw"] = np.ascontiguousarray(
            np.stack([aw[H:], aw[:H]], axis=1))                  # [128, 2]
        inm[f"g{l}_ab"] = np.full((H, 1), np.float32(w[pre + "_ab"]),
                                  dtype=np.float32)
    for l in range(2):
        wih = w[f"l{l}_Wih"][_GATE_PERM].copy()
        whh = w[f"l{l}_Whh"][_GATE_PERM].copy()
        bs = (w[f"l{l}_bih"] + w[f"l{l}_bhh"])[_GATE_PERM].copy()
        # tanh(g) = 2*sigmoid(2g) - 1: double the g-gate (block 3) params
        wih[384:512] *= 2.0
        whh[384:512] *= 2.0
        bs[384:512] *= 2.0
        wihT = np.ascontiguousarray(wih.T)                       # [128, 512]
        inm[f"l{l}_wihT"] = wihT if l == 0 else wihT.astype(bf16)
        inm[f"l{l}_whhT"] = np.ascontiguousarray(whh.T).astype(bf16)
        inm[f"l{l}_bsum"] = np.ascontiguousarray(
            bs.reshape(4, 128).T)                                # [128, 4]
    for hn, src, dout in (("op", "op", NUM_OPS), ("pa", "pa", NUM_PARAMS),
                          ("sk", "sk", SKETCH), ("no", "no", ND)):
        w1 = w[src + "_W1"]
        inm[f"{hn}_w1"] = w1 if hn == "no" else w1.astype(bf16)
        inm[f"{hn}_b1"] = w[src + "_b1"].reshape(H, 1)
        inm[f"{hn}_w2"] = w[src + "_W2"]
        b2 = w[src + "_b2"]
        nblk = (dout + 127) // 128
        inm[f"{hn}_b2"] = np.ascontiguousarray(
            b2.reshape(nblk, min(dout, 128)).T)                  # [<=128, nblk]
    return {k: (np.ascontiguousarray(v) if v.dtype == bf16 else
                np.ascontiguousarray(np.asarray(v, dtype=np.float32)))
            for k, v in inm.items()}


_CACHE = {}


def kernel(node_features, adjacency, mask, **w):
    node_features = np.asarray(node_features, dtype=np.float32)
    adjacency = np.asarray(adjacency, dtype=np.float32)
    w = {k: np.asarray(v, dtype=np.float32) for k, v in w.items()}

    if "nc" not in _CACHE:
        _CACHE["nc"] = build_program(N)
    nc = _CACHE["nc"]

    in_maps = [_prep_core_inputs(b, node_features, adjacency, w)
               for b in range(NCORES)]

    trace = bool(int(os.environ.get("BASSK_TRACE", "0")))
    res = bass_utils.run_bass_kernel_spmd(
        nc, in_maps, core_ids=list(range(NCORES)), trace=trace)
    if trace and res.exec_time_ns is not None:
        print(f"HW exec time: {res.exec_time_ns} ns")
    _CACHE["last_results"] = res

    op = np.stack([res.results[b]["op_out"] for b in range(B)])
    pa = np.stack([res.results[b]["pa_out"] for b in range(B)])
    sk = np.stack([res.results[b]["sk_out"] for b in range(B)])
    no = np.stack([res.results[b]["no_out"] for b in range(B)])
    return (op, pa, sk, no)


# revision 18
# speedup vs baseline: 2.3755x; 1.0940x over previous
"""Trainium2 Bass kernel for nn_DeepCAD (3x GNN-attention layers -> 2x LSTM -> 4 MLP heads).

Sharding: data-parallel over batch B=8 across the 8 NeuronCores (one batch
element per core); small weights replicated; host does the scatter/gather.

Per-core layout strategy:
  - GNN runs in "transposed" space: hT [H=128 partitions, N=1024 free].
  - attention: attT[j, i] = sigmoid(sj[j] + si[i] + ab) * adjT[j, i], built
    per 128-row chunk of j with sigmoid-bias = per-partition sj column and a
    PE-broadcast si row; att @ h via PE with lhsT = h (normal orientation,
    obtained by PE transposes of hT).
  - LSTM: x-parts precomputed as one big matmul into an interleaved
    [128, 4t+g] SBUF buffer; the 1024-step scan runs fully unrolled with
    gates-on-partitions [128, 4] tiles; gate col order (i, f, o, g) so one
    Sigmoid covers i,f,o and one Tanh covers g. Layer 1 is chunk-pipelined
    behind layer 0 (wavefront).
  - heads: chunked matmuls + PE transposes back to [t, dout] rows.
"""

import os
import numpy as np

import concourse.bass as bass
import concourse.tile as tile
from concourse import mybir
from concourse import bass_utils
from concourse.vector_clock import ScopedClock, VectorClock

F32 = mybir.dt.float32
FR = mybir.dt.bfloat16
AF = mybir.ActivationFunctionType
OP = mybir.AluOpType

B, N, ND, H = 8, 1024, 64, 128
NUM_OPS, NUM_PARAMS, SKETCH = 64, 256, 128
NCORES = 8

# gate permutation: torch order (i, f, g, o) -> our col order (i, f, o, g)
_GATE_PERM = np.concatenate([
    np.arange(0, 128),       # i
    np.arange(128, 256),     # f
    np.arange(384, 512),     # o
    np.arange(256, 384),     # g
])


# ---------------------------------------------------------------------------
# Workaround: this walrus encodes at most ONE sem-wait per Drain instruction.
# Tile's exit drain waits on every allocated sem at once -> split it.
def _drain_and_barrier_split(self, tick_clock, wait_clock):
    nc = self.nc
    vclock = tick_clock.global_clock
    emitted = False
    n = len(vclock)
    for p in range(n):
        t = vclock[p]
        if t <= 0:
            continue
        vc = VectorClock([0] * n)
        vc.require_at_least(p, t)
        d = nc.sync.drain()
        wait_clock.add_sem_waits(d.ins, ScopedClock({None: vc}))
        emitted = True
    if not emitted:
        nc.sync.drain()

    nc.all_engine_barrier()
    assert self.sems is not None
    popped = nc._tile_sem_poison_stack.pop()
    assert popped is self._sem_poison
    nc.clear_and_free_semaphores(list(self.sems.allocated().values()))
    nc.all_engine_barrier()


def _apply_tile_patch():
    tile.TileContext._drain_and_barrier = _drain_and_barrier_split


def _split_multi_waits(nc, limit=1):
    """This walrus encodes at most `limit` sem-waits per instruction; hoist
    extra waits onto same-engine NoOps inserted just before the instruction."""
    total = 0
    for f in nc.m.functions:
        for bb in f.blocks:
            insts = bb.instructions
            new_list = []
            for inst in insts:
                si = inst.sync_info
                if si is not None:
                    waits = list(si.on_wait)
                    if len(waits) > limit:
                        for w in waits[:-limit]:
                            nop = mybir.InstNoOp(name=f"waitsplit-{total}")
                            total += 1
                            nop.engine = inst.engine
                            nop.sync_info = mybir.SyncInfo(
                                on_wait=[w], on_update=[])
                            new_list.append(nop)
                        inst.sync_info = mybir.SyncInfo(
                            on_wait=waits[-limit:],
                            on_update=list(si.on_update))
                new_list.append(inst)
            if len(new_list) != len(insts):
                insts[:] = new_list
    return total


# ---------------------------------------------------------------------------
def build_program(n=N):
    """Build the per-core Bass program (SPMD: same program, per-core data)."""
    _apply_tile_patch()
    nc = bass.Bass("TRN2", target_bir_lowering=False, debug=False)

    nch = n // 128          # GNN j-chunks
    MM = min(512, n)        # moving-operand matmul width
    CH = min(128, n)        # LSTM emission chunk
    nlch = n // CH          # LSTM chunks
    HC = min(512, n)        # head chunk
    nhc = n // HC

    din0 = ND

    # ---- DRAM tensors -----------------------------------------------------
    def din_(name, shape):
        return nc.dram_tensor(name, list(shape), F32, kind="ExternalInput")

    xT_d = din_("xT", [din0, n])
    zeros_d = nc.dram_tensor("zeros1", [128, 1], FR, kind="ExternalInput")
    adjT_d = din_("adjT", [n, n])
    ident_d = din_("ident", [128, 128])

    gw_d, gb_d, gaw_d, gab_d = [], [], [], []
    for l in range(3):
        din = din0 if l == 0 else H
        gw_d.append(din_(f"g{l}_w", [din, H]))
        gb_d.append(din_(f"g{l}_b", [H, 1]))
        gaw_d.append(din_(f"g{l}_aw", [H, 2]))
        gab_d.append(din_(f"g{l}_ab", [H, 1]))

    wihT_d, whhT_d, bsum_d = [], [], []
    for l in range(2):
        wihT_d.append(nc.dram_tensor(f"l{l}_wihT", [H, 4 * H],
                                     F32 if l == 0 else FR,
                                     kind="ExternalInput"))
        whhT_d.append(nc.dram_tensor(f"l{l}_whhT", [H, 4 * H], FR,
                                     kind="ExternalInput"))
        bsum_d.append(din_(f"l{l}_bsum", [H, 4]))
    bsumr_d = [din_(f"l{l}_bsumr", [1, 4 * H]) for l in range(2)]

    HEADS = [("op", NUM_OPS), ("pa", NUM_PARAMS), ("sk", SKETCH), ("no", ND)]
    hw1_d, hb1_d, hw2_d, hb2_d, hout_d = {}, {}, {}, {}, {}
    for hn, dout in HEADS:
        hw1_d[hn] = nc.dram_tensor(f"{hn}_w1", [H, H],
                                   F32 if hn == "no" else FR,
                                   kind="ExternalInput")
        hb1_d[hn] = din_(f"{hn}_b1", [H, 1])
        hw2_d[hn] = din_(f"{hn}_w2", [H, dout])
        nblk = (dout + 127) // 128
        hb2_d[hn] = din_(f"{hn}_b2", [min(dout, 128), nblk])
        hout_d[hn] = nc.dram_tensor(f"{hn}_out", [n, dout], F32,
                                    kind="ExternalOutput")

    with tile.TileContext(nc) as tc:
        with (
            tc.tile_pool(name="const", bufs=1) as const,
            tc.tile_pool(name="work", bufs=2) as work,
            tc.tile_pool(name="att_p", bufs=3) as att_p,
            tc.tile_pool(name="small", bufs=4) as small,
        ):
            # ---- load constants/weights ----------------------------------
            ident = const.tile([128, 128], F32)
            nc.sync.dma_start(out=ident, in_=ident_d[:, :])
            ones_row = const.tile([1, 128], F32)
            nc.vector.memset(ones_row, 1.0)

            adj_sb = const.tile([128, nch, n], F32)
            for jc in range(nch):
                nc.sync.dma_start(out=adj_sb[:, jc, :],
                                  in_=adjT_d[jc * 128:(jc + 1) * 128, :])

            xT_sb = const.tile([din0, n], F32)
            nc.sync.dma_start(out=xT_sb, in_=xT_d[:, :])

            gw, gb, gaw, gab = [], [], [], []
            for l in range(3):
                din = din0 if l == 0 else H
                w = const.tile([din, H], F32, name=f"gw{l}")
                nc.sync.dma_start(out=w, in_=gw_d[l][:, :])
                b = const.tile([H, 1], F32, name=f"gb{l}")
                nc.sync.dma_start(out=b, in_=gb_d[l][:, :])
                aw = const.tile([H, 2], F32, name=f"gaw{l}")
                nc.sync.dma_start(out=aw, in_=gaw_d[l][:, :])
                ab = const.tile([H, 1], F32, name=f"gab{l}")
                nc.sync.dma_start(out=ab, in_=gab_d[l][:, :])
                gw.append(w); gb.append(b); gaw.append(aw); gab.append(ab)

            wihT, whhT, bsum = [], [], []
            for l in range(2):
                a = const.tile([H, 4 * H], F32 if l == 0 else FR,
                               name=f"wihT{l}")
                nc.sync.dma_start(out=a, in_=wihT_d[l][:, :])
                b_ = const.tile([H, 4 * H], FR, name=f"whhT{l}")
                nc.sync.dma_start(out=b_, in_=whhT_d[l][:, :])
                c_ = const.tile([H, 4], F32, name=f"bsum{l}")
                nc.sync.dma_start(out=c_, in_=bsum_d[l][:, :])
                wihT.append(a); whhT.append(b_); bsum.append(c_)
            bsum_row = []
            for l in range(2):
                br = const.tile([1, 4 * H], F32, name=f"bsumr{l}")
                nc.sync.dma_start(out=br, in_=bsumr_d[l][:, :])
                bsum_row.append(br)

            hw1, hb1, hw2, hb2 = {}, {}, {}, {}
            for hn, dout in HEADS:
                nblk = (dout + 127) // 128
                hw1[hn] = const.tile([H, H], F32 if hn == "no" else FR,
                                     name=f"hw1{hn}")
                nc.sync.dma_start(out=hw1[hn], in_=hw1_d[hn][:, :])
                hb1[hn] = const.tile([H, 1], F32, name=f"hb1{hn}")
                nc.sync.dma_start(out=hb1[hn], in_=hb1_d[hn][:, :])
                hw2[hn] = const.tile([H, dout], F32, name=f"hw2{hn}")
                nc.sync.dma_start(out=hw2[hn], in_=hw2_d[hn][:, :])
                hb2[hn] = const.tile([min(dout, 128), nblk], F32,
                                     name=f"hb2{hn}")
                nc.sync.dma_start(out=hb2[hn], in_=hb2_d[hn][:, :])

            # ---- GNN phase (own PSUM scope) ------------------------------
            cur = xT_sb
            din = din0
            hT3 = None
            with (
                tc.tile_pool(name="gps_big", bufs=2, space="PSUM") as gps_big,
                tc.tile_pool(name="gps_misc", bufs=2, space="PSUM") as gps_misc,
            ):
                for l in range(3):
                    # hT = W.T @ cur + b
                    hps = gps_big.tile([128, n], F32, tag="big")
                    for h0 in range(0, n, MM):
                        nc.tensor.matmul(hps[:, h0:h0 + MM], gw[l][:din, :],
                                         cur[:din, h0:h0 + MM],
                                         start=True, stop=True)
                    hT = work.tile([128, n], F32, tag=f"hT{l % 2}",
                                   name=f"hT{l}")
                    nc.vector.tensor_scalar(hT, hps, gb[l], None, OP.add)

                    # s = [sj; si] = aw.T @ hT
                    sps = gps_misc.tile([2, n], F32, tag="misc")
                    for h0 in range(0, n, MM):
                        nc.tensor.matmul(sps[:, h0:h0 + MM], gaw[l],
                                         hT[:, h0:h0 + MM],
                                         start=True, stop=True)
                    s_sb = work.tile([2, n], F32, tag="s_sb")
                    nc.vector.tensor_copy(s_sb, sps)

                    # si broadcast [128, n] via rank-1 PE matmul (stays in PSUM)
                    sib = gps_big.tile([128, n], F32, tag="big")
                    for h0 in range(0, n, MM):
                        nc.tensor.matmul(sib[:, h0:h0 + MM], ones_row,
                                         s_sb[0:1, h0:h0 + MM],
                                         start=True, stop=True)

                    outp = gps_big.tile([128, n], F32, tag="big")
                    hcols = work.tile([128, nch, 128], F32, tag="hcols")
                    for jc in range(nch):
                        js = slice(jc * 128, (jc + 1) * 128)
                        # sj column for this chunk (+ab)
                        tp = gps_misc.tile([128, 2], F32, tag="misc",
                                           name=f"tp{l}_{jc}")
                        nc.tensor.transpose(tp, s_sb[:, js], ident[0:2, 0:2])
                        sjab = small.tile([128, 1], F32, tag="sjab")
                        nc.vector.tensor_scalar(sjab, tp[:, 1:2], gab[l],
                                                None, OP.add)
                        # att chunk
                        att = att_p.tile([128, n], F32, tag="att")
                        nc.scalar.activation(att, sib, AF.Sigmoid, bias=sjab)
                        nc.vector.tensor_tensor(att, att, adj_sb[:, jc, :],
                                                OP.mult)
                        # h columns (normal orientation) for lhsT
                        hc = gps_misc.tile([128, 128], F32, tag="misc",
                                           name=f"hc{l}_{jc}")
                        nc.tensor.transpose(hc, hT[:, js], ident)
                        nc.vector.tensor_copy(hcols[:, jc, :], hc)
                        for h0 in range(0, n, MM):
                            nc.tensor.matmul(outp[:, h0:h0 + MM],
                                             hcols[:, jc, :],
                                             att[:, h0:h0 + MM],
                                             start=(jc == 0), stop=(jc == nch - 1))
                    nxt = work.tile([128, n], F32, tag=f"hT{(l + 1) % 2}",
                                    name=f"x{l + 1}")
                    if l < 2:
                        nc.vector.tensor_scalar(nxt, outp, 0.0, None, OP.max)
                    else:
                        nc.vector.tensor_copy(nxt, outp)
                    cur = nxt
                    din = H
                hT3 = cur

            # ---- LSTM + heads phase --------------------------------------
            lo0 = const.tile([128, n], FR)
            lo1 = const.tile([128, n], FR)

            zcol = const.tile([128, 1], FR)
            nc.sync.dma_start(out=zcol, in_=zeros_d[:, :])
            cst = [const.tile([128, 1], F32, name=f"c{l}") for l in range(2)]
            for l in range(2):
                nc.vector.memset(cst[l], 0.0)
            uacc = const.tile([128, 1], F32)

            with (
                tc.tile_pool(name="lps", bufs=3, space="PSUM") as lps,
                tc.tile_pool(name="gpch0", bufs=2, space="PSUM") as pch0,
                tc.tile_pool(name="gpch1", bufs=2, space="PSUM") as pch1,
                tc.tile_pool(name="wps", bufs=1, space="PSUM") as wps,
            ):
                pch = [pch0, pch1]
                psc_live = [None, None]
                _dbg_gate = [None, None]
                if int(os.environ.get("BASSK_DEBUG_LO", "0")):
                    g_dbg_d = nc.dram_tensor("g_dbg", [128, 8], F32,
                                             kind="ExternalOutput")
                    g_dbg = const.tile([128, 8], F32)
                    _dbg_gate[0] = g_dbg[:, 0:4]
                    _dbg_gate[1] = g_dbg[:, 4:8]
                    _dbg_gate.append((g_dbg_d, g_dbg))

                def prefill(l, k):
                    # per-chunk gate-psum bank: Wih.x + bias for CH steps,
                    # the per-step Whh.h matmuls accumulate on top.
                    ts0 = k * CH
                    src = hT3 if l == 0 else lo0
                    psc = pch[l].tile([128, 4, CH], F32, tag=f"pch{l}",
                                      name=f"pch{l}_{k}")
                    for g in range(4):
                        gs = slice(g * 128, (g + 1) * 128)
                        # start=True clears has_written for the WHOLE bank:
                        # only the first matmul touching it may set it.
                        nc.tensor.matmul(psc[:, g, :], wihT[l][:, gs],
                                         src[:, ts0:ts0 + CH],
                                         start=(g == 0), stop=False,
                                         skip_group_check=True)
                        nc.tensor.matmul(psc[:, g, :],
                                         bsum_row[l][:, gs],
                                         ones_row[:1, :CH],
                                         start=False, stop=False,
                                         skip_group_check=True)
                    psc_live[l] = psc

                # node head on hT3 (fills gaps during the LSTM scan)
                def head_chunk(hn, dout, src, k):
                    ts0 = k * HC
                    nblk = (dout + 127) // 128
                    m1ps = lps.tile([128, HC], F32, tag="cps",
                                    name=f"m1ps{hn}{k}")
                    nc.tensor.matmul(m1ps, hw1[hn], src[:, ts0:ts0 + HC],
                                     start=True, stop=True)
                    m1 = work.tile([128, HC], F32, tag="m1")
                    nc.vector.tensor_scalar(m1, m1ps, hb1[hn], 0.0,
                                            OP.add, OP.max)
                    for ob in range(nblk):
                        d = min(128, dout - ob * 128)
                        obs = slice(ob * 128, ob * 128 + d)
                        o2ps = lps.tile([128, HC], F32, tag="cps",
                                        name=f"o2ps{hn}{k}{ob}")
                        nc.tensor.matmul(o2ps[:d, :], hw2[hn][:, obs], m1,
                                         start=True, stop=True)
                        osb = work.tile([128, HC], F32, tag="osb")
                        nc.scalar.activation(osb[:d, :], o2ps[:d, :],
                                             AF.Identity,
                                             bias=hb2[hn][:d, ob:ob + 1])
                        tout = work.tile([128, (HC // 128) * 128], F32,
                                         tag="tout", name=f"tout{hn}{k}{ob}")
                        tov = tout.rearrange("p (tb d) -> p tb d", d=128)
                        for tb in range(HC // 128):
                            tps = lps.tile([128, 128], F32, tag="cps",
                                           name=f"tps{hn}{k}{ob}{tb}")
                            nc.tensor.transpose(
                                tps[:128, :d],
                                osb[:d, tb * 128:(tb + 1) * 128],
                                ident[:d, :d])
                            if ob % 2 == 0:
                                nc.vector.tensor_copy(tov[:, tb, :d],
                                                      tps[:128, :d])
                            else:
                                nc.scalar.copy(tov[:, tb, :d], tps[:128, :d])
                        dst = hout_d[hn][ts0:ts0 + HC, obs]
                        dstv = dst.rearrange("(tb p) d -> p tb d", p=128)
                        nc.sync.dma_start(out=dstv, in_=tov[:, :, :d])

                for k in range(nhc):
                    head_chunk("no", ND, hT3, k)

                # ---- the scan ----
                NWARM = int(os.environ.get("BASSK_WARM", "0"))

                def emit_warm(tag):
                    for i in range(NWARM):
                        wp = wps.tile([128, 512], F32, tag="warm",
                                      name=f"warm{tag}_{i}")
                        nc.tensor.matmul(wp[:1, :], ones_row[:1, :1],
                                         adj_sb[0:1, 0, 0:512],
                                         start=True, stop=True)

                # per-(layer, t) front half: gate matmuls accumulate onto
                # the prefetched x-part psum; one batched sigmoid (tanh(g)
                # via the doubled g-row trick: tanh(x) = 2*sigmoid(2x) - 1).
                s4_live = [None, None]

                def emit_h1(l, t, lo):
                    tau = t % CH
                    hprev = zcol if t == 0 else lo[:, t - 1:t]
                    psc = psc_live[l]
                    for g in range(4):
                        nc.tensor.matmul(
                            psc[:, g, tau:tau + 1],
                            whhT[l][:, g * 128:(g + 1) * 128],
                            hprev, start=False, stop=True,
                            skip_group_check=True)
                    if tau % 64 == 0:
                        emit_warm(f"{l}_{t}")
                    s4 = small.tile([128, 4], F32, tag=f"s4_{l}",
                                    name=f"s4_{l}_{t}")
                    nc.scalar.activation(s4, psc[:, :, tau], AF.Sigmoid)
                    s4_live[l] = s4
                    if t == 0 and l == 0 and _dbg_gate[0] is not None:
                        nc.vector.tensor_copy(_dbg_gate[0], psc[:, :, tau])
                        nc.vector.tensor_copy(_dbg_gate[1], s4)

                # back half: state update + output
                def emit_h2(l, t, lo):
                    s4 = s4_live[l]
                    u1 = small.tile([128, 1], F32, tag=f"u1{l}",
                                    name=f"u1{l}_{t}")
                    # u = (2*sg~ - 1)*si = si*tanh(g): u1 = 2*sg~*si, u = u1-si
                    nc.vector.scalar_tensor_tensor(
                        u1, s4[:, 3:4], 2.0, s4[:, 0:1], OP.mult, OP.mult)
                    u = small.tile([128, 1], F32, tag=f"u{l}",
                                   name=f"u{l}_{t}")
                    nc.vector.tensor_tensor(u, u1, s4[:, 0:1], OP.subtract)
                    # c = c*sig_f + u   (in place)
                    nc.vector.scalar_tensor_tensor(
                        cst[l], cst[l], s4[:, 1:2], u, OP.mult, OP.add)
                    tc_ = small.tile([128, 1], F32, tag=f"tc{l}",
                                     name=f"tc{l}_{t}")
                    nc.scalar.activation(tc_, cst[l], AF.Tanh)
                    nc.vector.tensor_scalar(lo[:, t:t + 1], tc_,
                                            s4[:, 2:3], None, OP.mult)

                def lstm_chunk(l, k, lo):
                    prefill(l, k)
                    for tau in range(CH):
                        t = k * CH + tau
                        emit_h1(l, t, lo)
                        emit_h2(l, t, lo)

                lstm_chunk(0, 0, lo0)
                for k in range(1, nlch):
                    a = k * CH
                    b = (k - 1) * CH
                    prefill(0, k)
                    prefill(1, k - 1)
                    ps0, ps1 = psc_live[0], psc_live[1]
                    for tau in range(CH):
                        psc_live[0] = ps0
                        emit_h1(0, a + tau, lo0)
                        if b + tau - 1 >= 0:
                            emit_h2(1, b + tau - 1, lo1)
                        emit_h2(0, a + tau, lo0)
                        psc_live[1] = ps1
                        emit_h1(1, b + tau, lo1)
                # epilogue: l1's last chunk
                b = (nlch - 1) * CH
                prefill(1, nlch - 1)
                for tau in range(CH):
                    if b + tau - 1 >= 0:
                        emit_h2(1, b + tau - 1, lo1)
                    emit_h1(1, b + tau, lo1)
                emit_h2(1, n - 1, lo1)

                # op/pa/sk heads on lo1
                for k in range(nhc):
                    for hn, dout in HEADS[:3]:
                        head_chunk(hn, dout, lo1, k)

                if int(os.environ.get("BASSK_DEBUG_LO", "0")):
                    nc.sync.dma_start(out=_dbg_gate[2][0][:, :],
                                      in_=_dbg_gate[2][1])
                    lo0_d = nc.dram_tensor("lo0_dbg", [128, n], F32,
                                           kind="ExternalOutput")
                    lo1_d = nc.dram_tensor("lo1_dbg", [128, n], F32,
                                           kind="ExternalOutput")
                    lo0f = work.tile([128, n], F32, tag="dbg0")
                    nc.vector.tensor_copy(lo0f, lo0)
                    nc.sync.dma_start(out=lo0_d[:, :], in_=lo0f)
                    lo1f = work.tile([128, n], F32, tag="dbg1")
                    nc.vector.tensor_copy(lo1f, lo1)
                    nc.sync.dma_start(out=lo1_d[:, :], in_=lo1f)

    nsplit = _split_multi_waits(nc)
    if int(os.environ.get("BASSK_VERBOSE", "0")):
        print(f"build: split {nsplit} extra waits")
    return nc


# ---------------------------------------------------------------------------
def _prep_core_inputs(b, node_features, adjacency, weights, n=N):
    """Build the per-core input map (numpy) for batch element b."""
    w = weights
    import ml_dtypes
    bf16 = ml_dtypes.bfloat16
    inm = {
        "zeros1": np.zeros((128, 1), bf16),
        "xT": np.ascontiguousarray(node_features[b].T),          # [64, n]
        "adjT": np.ascontiguousarray(adjacency[b].T),            # [n, n]
        "ident": np.eye(128, dtype=np.float32),
    }
    for l, pre in enumerate(("g1", "g2", "g3")):
        inm[f"g{l}_w"] = w[pre + "_W"]
        inm[f"g{l}_b"] = w[pre + "_b"].reshape(H, 1)
        aw = w[pre + "_aW"]
        # col 0 = si weights (aW[H:]), col 1 = sj weights (aW[:H]) so the
        # broadcast row (si) sits at base partition 0 of s
        inm[f"g{l}_aw"] = np.ascontiguousarray(
            np.stack([aw[H:], aw[:H]], axis=1))                  # [128, 2]
        inm[f"g{l}_ab"] = np.full((H, 1), np.float32(w[pre + "_ab"]),
                                  dtype=np.float32)
    for l in range(2):
        wih = w[f"l{l}_Wih"][_GATE_PERM].copy()
        whh = w[f"l{l}_Whh"][_GATE_PERM].copy()
        bs = (w[f"l{l}_bih"] + w[f"l{l}_bhh"])[_GATE_PERM].copy()
        # tanh(g) = 2*sigmoid(2g) - 1: double the g-gate (block 3) params
        wih[384:512] *= 2.0
        whh[384:512] *= 2.0
        bs[384:512] *= 2.0
        wihT = np.ascontiguousarray(wih.T)                       # [128, 512]
        inm[f"l{l}_wihT"] = wihT if l == 0 else wihT.astype(bf16)
        inm[f"l{l}_whhT"] = np.ascontiguousarray(whh.T).astype(bf16)
        inm[f"l{l}_bsum"] = np.ascontiguousarray(
            bs.reshape(4, 128).T)                                # [128, 4]
        inm[f"l{l}_bsumr"] = np.ascontiguousarray(bs.reshape(1, 512))
    for hn, src, dout in (("op", "op", NUM_OPS), ("pa", "pa", NUM_PARAMS),
                          ("sk", "sk", SKETCH), ("no", "no", ND)):
        w1 = w[src + "_W1"]
        inm[f"{hn}_w1"] = w1 if hn == "no" else w1.astype(bf16)
        inm[f"{hn}_b1"] = w[src + "_b1"].reshape(H, 1)
        inm[f"{hn}_w2"] = w[src + "_W2"]
        b2 = w[src + "_b2"]
        nblk = (dout + 127) // 128
        inm[f"{hn}_b2"] = np.ascontiguousarray(
            b2.reshape(nblk, min(dout, 128)).T)                  # [<=128, nblk]
    return {k: (np.ascontiguousarray(v) if v.dtype == bf16 else
                np.ascontiguousarray(np.asarray(v, dtype=np.float32)))
            for k, v in inm.items()}


_CACHE = {}


def kernel(node_features, adjacency, mask, **w):
    node_features = np.asarray(node_features, dtype=np.float32)
    adjacency = np.asarray(adjacency, dtype=np.float32)
    w = {k: np.asarray(v, dtype=np.float32) for k, v in w.items()}

    if "nc" not in _CACHE:
        _CACHE["nc"] = build_program(N)
    nc = _CACHE["nc"]

    in_maps = [_prep_core_inputs(b, node_features, adjacency, w)
               for b in range(NCORES)]

    trace = bool(int(os.environ.get("BASSK_TRACE", "0")))
    res = bass_utils.run_bass_kernel_spmd(
        nc, in_maps, core_ids=list(range(NCORES)), trace=trace)
    if trace and res.exec_time_ns is not None:
        print(f"HW exec time: {res.exec_time_ns} ns")
    _CACHE["last_results"] = res

    op = np.stack([res.results[b]["op_out"] for b in range(B)])
    pa = np.stack([res.results[b]["pa_out"] for b in range(B)])
    sk = np.stack([res.results[b]["sk_out"] for b in range(B)])
    no = np.stack([res.results[b]["no_out"] for b in range(B)])
    return (op, pa, sk, no)
